# revision 1
# baseline (speedup 1.0000x reference)
"""Trainium2 Bass kernel for nn_Attention_65223373357517.

Computes, for s,q [B=16, L=1024, D=1024] (D = 2H, H=512):
    a  = einsum('bsd,btd->bst', s, q)
    b  = softmax(a, -1) @ q
    c  = softmax(a^T, -1) @ s
    s~ = heuristic(s, b);  q~ = heuristic(q, c)
with heuristic(x, y) = g*r + (1-g)*x,
    r = gelu_tanh([x, y, x*y, x-y] @ w_r.T + b_r)
    g = sigmoid ([x, y, x*y, x-y] @ w_g.T + b_g)

Strategy: pure data-parallel over batch (2 examples per NeuronCore, 8 cores,
no collectives). Host folds the (x-y) block into the x/y weight blocks
(W1+W4, W2-W4, W3), transposes activations so every on-chip matmul is in
its natural layout, and transposes outputs back.  Masks are all-ones in
this problem configuration (additive mask term is identically zero), so
they do not enter the computation.

On-chip per batch:
  stage 1: A = S Q^T via float32r matmuls (full PE speed, ~1e-4 precision),
           A kept in SBUF f32; row stats m1, d1 = sum exp(A - m1) via
           fused ACT exp+accum; l1 = m1 + ln d1.
  stage T: A^T via PE transposes into PSUM; row stats m2/d2 of A^T;
           P1^T = exp(A^T - l1[s]) with the free-dim shift done by
           gpsimd.partition_broadcast + DVE subtract; bf16.
  stage 2: b^T = Q_nat^T-contracted matmul with rhs P1^T (bf16);
           P2^T = exp(A - l2[t]); c^T similarly with lhsT = S_nat.
  heur:    per 128-row output strip: 24 K-chunk bf16 matmuls each for the
           r and g branches over blocks [x^T, y^T, (x*y)^T]; gelu/sigmoid
           read PSUM directly with per-partition bias; epilogue
           out = x + g*(r - x) on DVE/GPSIMD; stream out s~^T / q~^T.
"""

import numpy as np
import ml_dtypes

B, L, D = 16, 1024, 1024
NCORES = 8
BLOC = B // NCORES          # batches per core
NK = D // 128               # contraction chunks for stage 1/2
NM = D // 128               # output-row chunks
KF = 3 * D // 128           # folded heuristic contraction chunks (24)
NH = 2                      # 512-wide halves of a 1024 free dim

_nc_cache = None


def _build():
    import concourse.tile as tile
    from concourse import bacc, mybir

    FP32 = mybir.dt.float32
    FP32R = mybir.dt.float32r
    I32 = mybir.dt.int32
    BF16 = mybir.dt.bfloat16
    AF = mybir.ActivationFunctionType
    ALU = mybir.AluOpType
    AX = mybir.AxisListType

    nc = bacc.Bacc("TRN2", target_bir_lowering=False, debug=False)

    st_d = nc.dram_tensor("st", [BLOC, D, L], FP32R, kind="ExternalInput")
    qt_d = nc.dram_tensor("qt", [BLOC, D, L], FP32R, kind="ExternalInput")
    snb_d = nc.dram_tensor("snb", [BLOC, L, D], BF16, kind="ExternalInput")
    qnb_d = nc.dram_tensor("qnb", [BLOC, L, D], BF16, kind="ExternalInput")
    stb_d = nc.dram_tensor("stb", [BLOC, D, L], BF16, kind="ExternalInput")
    qtb_d = nc.dram_tensor("qtb", [BLOC, D, L], BF16, kind="ExternalInput")
    wr_d = nc.dram_tensor("wr", [NM, 128, KF, 128], BF16, kind="ExternalInput")
    wg_d = nc.dram_tensor("wg", [NM, 128, KF, 128], BF16, kind="ExternalInput")
    brt_d = nc.dram_tensor("brt", [128, NM], FP32, kind="ExternalInput")
    bgt_d = nc.dram_tensor("bgt", [128, NM], FP32, kind="ExternalInput")
    outs_d = nc.dram_tensor("outs", [BLOC, D, L], FP32, kind="ExternalOutput")
    outq_d = nc.dram_tensor("outq", [BLOC, D, L], FP32, kind="ExternalOutput")
    ident_d = nc.inline_tensor(np.eye(128, dtype=np.float32), name="identsrc")

    with tile.TileContext(nc) as tc:
        with (
            tc.tile_pool(name="prog", bufs=1) as Pp,
            tc.tile_pool(name="qpool", bufs=1) as Pq,
            tc.tile_pool(name="lpsum", bufs=1, space="PSUM") as PSl,
        ):
            ident = Pp.tile([128, 128], FP32, tag="ident", name="ident")
            nc.sync.dma_start(ident[:], ident_d[:])
            brt = Pp.tile([128, NM], FP32, tag="brt", name="brt")
            nc.sync.dma_start(brt[:], brt_d[:])
            bgt = Pp.tile([128, NM], FP32, tag="bgt", name="bgt")
            nc.sync.dma_start(bgt[:], bgt_d[:])

            def load_qtf(b, h):
                ts = []
                for k in range(NK):
                    t = Pq.tile([128, 512], FP32R, tag="qtf", bufs=NK,
                                name=f"qtf{b}_{h}_{k}")
                    nc.sync.dma_start(
                        t[:], qt_d[b, k * 128:(k + 1) * 128,
                                   h * 512:(h + 1) * 512])
                    ts.append(t)
                return ts

            qtf_pre = load_qtf(0, 0)

            for b in range(BLOC):
                with tc.tile_pool(name=f"long{b}", bufs=1) as Pl:
                    stbt = [Pl.tile([128, L], BF16, tag="stb", bufs=NK,
                                    name=f"stb{b}_{k}") for k in range(NK)]
                    qtbt = [Pl.tile([128, L], BF16, tag="qtb", bufs=NK,
                                    name=f"qtb{b}_{k}") for k in range(NK)]
                    negm1 = Pl.tile([128, NK], FP32, tag="negm1", name=f"negm1{b}")
                    d1 = Pl.tile([128, NK], FP32, tag="d1", name=f"d1{b}")
                    l1a = Pl.tile([128, NK], FP32, tag="l1a", name=f"l1a{b}")
                    negm2 = Pl.tile([128, NK], FP32, tag="negm2", name=f"negm2{b}")
                    d2 = Pl.tile([128, NK], FP32, tag="d2", name=f"d2{b}")
                    l2a = Pl.tile([128, NK], FP32, tag="l2a", name=f"l2a{b}")
                    lt8 = Pl.tile([8, 128], FP32, tag="lt8", name=f"lt8{b}")
                    l1row = Pl.tile([1, L], FP32, tag="l1row", name=f"l1row{b}")
                    l2row = Pl.tile([1, L], FP32, tag="l2row", name=f"l2row{b}")
                    bT = []
                    cT = []

                    with tc.tile_pool(name=f"apool{b}", bufs=1) as Pa:
                        A = [Pa.tile([128, L], FP32, tag="A", bufs=NK,
                                     name=f"A{b}_{ms}") for ms in range(NK)]
                        with (
                            tc.tile_pool(name=f"s1{b}", bufs=1) as P1,
                            tc.tile_pool(name=f"ps1{b}", bufs=4, space="PSUM") as PS1,
                        ):
                            # stage 1: A = S Q^T (f32r) one 512-half at a time
                            for h in range(NH):
                                qtf = qtf_pre if (h == 0) else load_qtf(b, 1)
                                for ms in range(NK):
                                    pa = PS1.tile([128, 512], FP32, tag="pa",
                                                  bufs=4, name=f"pa{b}_{h}_{ms}")
                                    for k in range(NK):
                                        stf = P1.tile(
                                            [128, 128], FP32R, tag="stf", bufs=4,
                                            name=f"stf{b}_{h}_{ms}_{k}")
                                        nc.sync.dma_start(
                                            stf[:],
                                            st_d[b, k * 128:(k + 1) * 128,
                                                 ms * 128:(ms + 1) * 128])
                                        nc.tensor.matmul(
                                            pa[:], stf[:], qtf[k][:],
                                            start=(k == 0), stop=(k == NK - 1))
                                    nc.vector.tensor_copy(
                                        A[ms][:, h * 512:(h + 1) * 512], pa[:])
                                    if h == 1:
                                        nc.vector.tensor_reduce(
                                            negm1[:, ms:ms + 1], A[ms][:], AX.X,
                                            ALU.max, negate=True)
                                        esc = P1.tile(
                                            [128, L], BF16, tag="escr", bufs=2,
                                            name=f"escr{b}_{ms}")
                                        nc.scalar.activation(
                                            esc[:], A[ms][:], AF.Exp,
                                            bias=negm1[:, ms:ms + 1],
                                            accum_out=d1[:, ms:ms + 1])
                            # l1 = m1 + ln d1
                            lnd = P1.tile([128, NK], FP32, tag="lnd",
                                          name=f"lnd{b}")
                            nc.scalar.activation(lnd[:], d1[:], AF.Ln)
                            nc.vector.tensor_sub(l1a[:], lnd[:], negm1[:])

                        with (
                            tc.tile_pool(name=f"T{b}", bufs=1) as Pt,
                            tc.tile_pool(name=f"psT{b}", bufs=2, space="PSUM") as PSt,
                        ):
                            # prefetches that overlap the softmax phase
                            for k in range(NK):
                                nc.sync.dma_start(
                                    stbt[k][:], stb_d[b, k * 128:(k + 1) * 128, :])
                                nc.sync.dma_start(
                                    qtbt[k][:], qtb_d[b, k * 128:(k + 1) * 128, :])
                            qnr = []
                            for k in range(NK):
                                tq = Pt.tile([128, D], BF16, tag="nat", bufs=NK,
                                             name=f"qnr{b}_{k}")
                                nc.sync.dma_start(
                                    tq[:], qnb_d[b, k * 128:(k + 1) * 128, :])
                                qnr.append(tq)
                            if b + 1 < BLOC:
                                qtf_pre = load_qtf(b + 1, 0)

                            # l1 broadcast: [128, NK] -> [1, L] -> [128, L]
                            lp1 = PSl.tile([8, 128], FP32, tag="lp", bufs=1,
                                           name=f"lp1{b}")
                            nc.tensor.transpose(lp1[:], l1a[:], ident[:])
                            nc.vector.tensor_copy(lt8[:], lp1[:])
                            nc.sync.dma_start(
                                l1row[:1, :].rearrange("p (c f) -> p c f", f=128),
                                lt8[:])
                            l1bc = Pt.tile([128, L], FP32, tag="l1bc",
                                           name=f"l1bc{b}")
                            nc.gpsimd.partition_broadcast(l1bc[:], l1row[:])

                            # A^T tiles -> m2/d2 stats and P1^T = exp(A^T - l1)
                            p1t = []
                            for mt in range(NK):
                                at = PSt.tile([128, L], FP32, tag="at", bufs=2,
                                              name=f"at{b}_{mt}")
                                for c in range(NK):
                                    nc.tensor.transpose(
                                        at[:, c * 128:(c + 1) * 128],
                                        A[c][:, mt * 128:(mt + 1) * 128],
                                        ident[:])
                                nc.vector.tensor_reduce(
                                    negm2[:, mt:mt + 1], at[:], AX.X, ALU.max,
                                    negate=True)
                                e2 = Pt.tile([128, L], BF16, tag="e2scr", bufs=1,
                                             name=f"e2{b}_{mt}")
                                nc.scalar.activation(
                                    e2[:], at[:], AF.Exp,
                                    bias=negm2[:, mt:mt + 1],
                                    accum_out=d2[:, mt:mt + 1])
                                sh = Pt.tile([128, L], FP32, tag="shift", bufs=2,
                                             name=f"sh{b}_{mt}")
                                nc.vector.tensor_sub(sh[:], at[:], l1bc[:])
                                pt_ = Pt.tile([128, L], BF16, tag="p1t", bufs=NK,
                                              name=f"p1t{b}_{mt}")
                                nc.scalar.activation(pt_[:], sh[:], AF.Exp)
                                p1t.append(pt_)

                            # l2 = m2 + ln d2 -> row -> broadcast
                            lnd2 = Pt.tile([128, NK], FP32, tag="lnd2",
                                           name=f"lnd2{b}")
                            nc.scalar.activation(lnd2[:], d2[:], AF.Ln)
                            nc.vector.tensor_sub(l2a[:], lnd2[:], negm2[:])
                            lp2 = PSl.tile([8, 128], FP32, tag="lp", bufs=1,
                                           name=f"lp2{b}")
                            nc.tensor.transpose(lp2[:], l2a[:], ident[:])
                            nc.vector.tensor_copy(lt8[:], lp2[:])
                            nc.sync.dma_start(
                                l2row[:1, :].rearrange("p (c f) -> p c f", f=128),
                                lt8[:])

                            # b^T = sum_t Q_nat[t,d] P1^T[t,s]
                            for md in range(NM):
                                pb = [PSt.tile([128, 512], FP32, tag="pb", bufs=2,
                                               name=f"pb{b}_{md}_{h}")
                                      for h in range(NH)]
                                for kt in range(NK):
                                    for h in range(NH):
                                        nc.tensor.matmul(
                                            pb[h][:],
                                            qnr[kt][:, md * 128:(md + 1) * 128],
                                            p1t[kt][:, h * 512:(h + 1) * 512],
                                            start=(kt == 0), stop=(kt == NK - 1))
                                bt_ = Pl.tile([128, L], BF16, tag="bT", bufs=NM,
                                              name=f"bT{b}_{md}")
                                for h in range(NH):
                                    nc.vector.tensor_copy(
                                        bt_[:, h * 512:(h + 1) * 512], pb[h][:])
                                bT.append(bt_)

                            # P2^T = exp(A - l2); c lhsT reuses the qn slots
                            l2bc = Pt.tile([128, L], FP32, tag="l2bc",
                                           name=f"l2bc{b}")
                            nc.gpsimd.partition_broadcast(l2bc[:], l2row[:])
                            snr = []
                            for k in range(NK):
                                ts_ = Pt.tile([128, D], BF16, tag="nat", bufs=NK,
                                              name=f"snr{b}_{k}")
                                nc.sync.dma_start(
                                    ts_[:], snb_d[b, k * 128:(k + 1) * 128, :])
                                snr.append(ts_)
                            p2t = []
                            for c in range(NK):
                                sh = Pt.tile([128, L], FP32, tag="shift", bufs=2,
                                             name=f"sh2{b}_{c}")
                                nc.vector.tensor_sub(sh[:], A[c][:], l2bc[:])
                                pt_ = Pt.tile([128, L], BF16, tag="p2t", bufs=NK,
                                              name=f"p2t{b}_{c}")
                                nc.scalar.activation(pt_[:], sh[:], AF.Exp)
                                p2t.append(pt_)

                            # c^T = sum_s S_nat[s,d] P2^T[s,t]
                            for md in range(NM):
                                pb = [PSt.tile([128, 512], FP32, tag="pb", bufs=2,
                                               name=f"pc{b}_{md}_{h}")
                                      for h in range(NH)]
                                for ks in range(NK):
                                    for h in range(NH):
                                        nc.tensor.matmul(
                                            pb[h][:],
                                            snr[ks][:, md * 128:(md + 1) * 128],
                                            p2t[ks][:, h * 512:(h + 1) * 512],
                                            start=(ks == 0), stop=(ks == NK - 1))
                                ct_ = Pl.tile([128, L], BF16, tag="cT", bufs=NM,
                                              name=f"cT{b}_{md}")
                                for h in range(NH):
                                    nc.vector.tensor_copy(
                                        ct_[:, h * 512:(h + 1) * 512], pb[h][:])
                                cT.append(ct_)

                    # heuristic for (x=s, y=b) -> outs and (x=q, y=c) -> outq
                    with (
                        tc.tile_pool(name=f"heur{b}", bufs=1) as Ph,
                        tc.tile_pool(name=f"psH{b}", bufs=7, space="PSUM") as PSh,
                    ):
                        xys = []
                        xyq = []
                        for k in range(NK):
                            t1 = Ph.tile([128, L], BF16, tag="xys", bufs=NK,
                                         name=f"xys{b}_{k}")
                            nc.vector.tensor_mul(t1[:], stbt[k][:], bT[k][:])
                            xys.append(t1)
                            t2 = Ph.tile([128, L], BF16, tag="xyq", bufs=NK,
                                         name=f"xyq{b}_{k}")
                            nc.vector.tensor_mul(t2[:], qtbt[k][:], cT[k][:])
                            xyq.append(t2)

                        for m in range(NM):
                            wrt = Ph.tile([128, KF, 128], BF16, tag="wr", bufs=2,
                                          name=f"wrt{b}_{m}")
                            nc.sync.dma_start(wrt[:], wr_d[m])
                            wgt = Ph.tile([128, KF, 128], BF16, tag="wg", bufs=2,
                                          name=f"wgt{b}_{m}")
                            nc.sync.dma_start(wgt[:], wg_d[m])
                            for xt, blocks, outd in (
                                (stbt, (stbt, bT, xys), outs_d),
                                (qtbt, (qtbt, cT, xyq), outq_d),
                            ):
                                tag = "s" if outd is outs_d else "q"
                                pr = [PSh.tile([128, 512], FP32, tag="rg", bufs=7,
                                               name=f"pr{b}_{m}{tag}{h}")
                                      for h in range(NH)]
                                pg = [PSh.tile([128, 512], FP32, tag="rg", bufs=7,
                                               name=f"pg{b}_{m}{tag}{h}")
                                      for h in range(NH)]
                                for kf in range(KF):
                                    rhs = blocks[kf // NK][kf % NK]
                                    for h in range(NH):
                                        nc.tensor.matmul(
                                            pr[h][:], wrt[:, kf, :],
                                            rhs[:, h * 512:(h + 1) * 512],
                                            start=(kf == 0), stop=(kf == KF - 1))
                                    for h in range(NH):
                                        nc.tensor.matmul(
                                            pg[h][:], wgt[:, kf, :],
                                            rhs[:, h * 512:(h + 1) * 512],
                                            start=(kf == 0), stop=(kf == KF - 1))
                                r_sb = Ph.tile([128, L], BF16, tag="rsb", bufs=2,
                                               name=f"rsb{b}_{m}{tag}")
                                g_sb = Ph.tile([128, L], BF16, tag="gsb", bufs=2,
                                               name=f"gsb{b}_{m}{tag}")
                                for h in range(NH):
                                    nc.scalar.activation(
                                        r_sb[:, h * 512:(h + 1) * 512], pr[h][:],
                                        AF.Gelu_apprx_tanh, bias=brt[:, m:m + 1])
                                for h in range(NH):
                                    nc.scalar.activation(
                                        g_sb[:, h * 512:(h + 1) * 512], pg[h][:],
                                        AF.Sigmoid, bias=bgt[:, m:m + 1])
                                t1 = Ph.tile([128, L], FP32, tag="t1", bufs=2,
                                             name=f"t1{b}_{m}{tag}")
                                nc.vector.tensor_sub(t1[:], r_sb[:], xt[m][:])
                                t2 = Ph.tile([128, L], FP32, tag="t2", bufs=2,
                                             name=f"t2{b}_{m}{tag}")
                                nc.gpsimd.tensor_mul(t2[:], g_sb[:], t1[:])
                                osb = Ph.tile([128, L], FP32, tag="osb", bufs=2,
                                              name=f"osb{b}_{m}{tag}")
                                nc.vector.tensor_add(osb[:], t2[:], xt[m][:])
                                nc.sync.dma_start(
                                    outd[b, m * 128:(m + 1) * 128, :], osb[:])

    nc.compile()
    return nc


def _get_nc():
    global _nc_cache
    if _nc_cache is None:
        _nc_cache = _build()
    return _nc_cache


def _prep_inputs(s, q, w_r, b_r, w_g, b_g):
    bf = ml_dtypes.bfloat16
    s = np.ascontiguousarray(np.asarray(s, dtype=np.float32))
    q = np.ascontiguousarray(np.asarray(q, dtype=np.float32))
    w_r = np.asarray(w_r, dtype=np.float32)
    w_g = np.asarray(w_g, dtype=np.float32)
    b_r = np.asarray(b_r, dtype=np.float32)
    b_g = np.asarray(b_g, dtype=np.float32)

    st = np.ascontiguousarray(s.transpose(0, 2, 1))
    qt = np.ascontiguousarray(q.transpose(0, 2, 1))
    snb = s.astype(bf)
    qnb = q.astype(bf)
    stb = st.astype(bf)
    qtb = qt.astype(bf)

    def pack_w(w):
        W1, W2, W3, W4 = (w[:, i * D:(i + 1) * D] for i in range(4))
        eff = np.concatenate([W1 + W4, W2 - W4, W3], axis=1)  # [D, 3D]
        wt = eff.T  # [3D, D]
        pk = wt.reshape(KF, 128, NM, 128).transpose(2, 1, 0, 3)  # [m, f, k, o]
        return np.ascontiguousarray(pk).astype(bf)

    wr_pack = pack_w(w_r)
    wg_pack = pack_w(w_g)
    brt = np.ascontiguousarray(b_r.reshape(NM, 128).T)
    bgt = np.ascontiguousarray(b_g.reshape(NM, 128).T)

    in_maps = []
    for c in range(NCORES):
        sl = slice(BLOC * c, BLOC * (c + 1))
        in_maps.append({
            "st": st[sl], "qt": qt[sl],
            "snb": snb[sl], "qnb": qnb[sl],
            "stb": stb[sl], "qtb": qtb[sl],
            "wr": wr_pack, "wg": wg_pack,
            "brt": brt, "bgt": bgt,
        })
    return in_maps


def run(inputs, trace=False, tmpdir=None):
    """Execute on 8 NeuronCores; returns ((s_tilde, q_tilde), BassKernelResults)."""
    from concourse.bass_utils import run_bass_kernel_spmd

    in_maps = _prep_inputs(
        inputs["s"], inputs["q"], inputs["w_r"], inputs["b_r"],
        inputs["w_g"], inputs["b_g"])
    nc = _get_nc()
    res = run_bass_kernel_spmd(nc, in_maps, list(range(NCORES)), trace=trace,
                               tmpdir=tmpdir)
    s_t = np.empty((B, L, D), np.float32)
    q_t = np.empty((B, L, D), np.float32)
    for c in range(NCORES):
        sl = slice(BLOC * c, BLOC * (c + 1))
        s_t[sl] = res.results[c]["outs"].transpose(0, 2, 1)
        q_t[sl] = res.results[c]["outq"].transpose(0, 2, 1)
    return (s_t, q_t), res


def kernel(s, q, w_r, b_r, w_g, b_g, s_mask=None, q_mask=None):
    # s_mask / q_mask are all-ones in this problem; the additive mask term
    # (1 - m1*m2) * NEG_INF is identically zero, so they are unused.
    out, _ = run({"s": s, "q": q, "w_r": w_r, "b_r": b_r,
                  "w_g": w_g, "b_g": b_g})
    return out



# revision 8
# speedup vs baseline: 1.1444x; 1.1444x over previous
"""Trainium2 Bass kernel for nn_Attention_65223373357517.

Computes, for s,q [B=16, L=1024, D=1024] (D = 2H, H=512):
    a  = einsum('bsd,btd->bst', s, q)
    b  = softmax(a, -1) @ q
    c  = softmax(a^T, -1) @ s
    s~ = heuristic(s, b);  q~ = heuristic(q, c)
with heuristic(x, y) = g*r + (1-g)*x,
    r = gelu_tanh([x, y, x*y, x-y] @ w_r.T + b_r)
    g = sigmoid ([x, y, x*y, x-y] @ w_g.T + b_g)

Strategy: pure data-parallel over batch (2 examples per NeuronCore, 8 cores,
no collectives). Host folds the (x-y) block into the x/y weight blocks
(W1+W4, W2-W4, W3), transposes activations so every on-chip matmul is in
its natural layout, and transposes outputs back.  Masks are all-ones in
this problem configuration (additive mask term is identically zero), so
they do not enter the computation.

On-chip per batch (v2 schedule — PE kept saturated):
  S1:  A = S Q^T via f32r matmuls.  qt rows resident (prefetched during
       the previous batch's heuristic); st streamed as [128,128]
       stationary chunks, each reused for both 512-halves.  ms processed
       in waves of 3 (PSUM bound), k inner so compute chases DMA.
       Per A tile: row max m1, d1 = sum exp(A-m1) (fused ACT accum),
       l1 = m1 + ln d1 folded into the ACT bias so P1 = exp(A - l1) is
       the *normalized* softmax in A layout (bf16, per-partition bias —
       no cross-partition broadcast needed).  PE bf16-transposes P1 into
       p1t strips, issued one wave late so the ACT chain stays hidden.
  B/C: A^T tiles via PE fp32 transposes (PSUM); same per-tile stat chain
       gives P2^T = exp(A^T - l2) bf16, transposed back into p2 strips.
       b^T = Q_nat-chunk-contracted matmuls with rhs p1t; c^T likewise
       with lhsT = S_nat chunks, rhs p2.  at/p2 transposes interleave
       with the b^T matmul stream so PE never waits on stat chains.
  H:   heuristic per 128-row output strip: 24 K-chunk bf16 matmuls each
       for r and g over blocks [x^T, y^T, (x*y)^T]; gelu/sigmoid read
       PSUM with per-partition bias; epilogue out = x + g*(r - x);
       stream out s~^T / q~^T.  Next batch's qt prefetches here.
"""

import numpy as np
import ml_dtypes

B, L, D = 16, 1024, 1024
NCORES = 8
BLOC = B // NCORES          # batches per core
NK = D // 128               # contraction chunks (8)
NM = D // 128               # output-row chunks (8)
KF = 3 * D // 128           # folded heuristic contraction chunks (24)
NH = 2                      # 512-wide halves of a 1024 free dim

_nc_cache = None


def _build():
    import concourse.tile as tile
    from concourse import bacc, mybir

    FP32 = mybir.dt.float32
    FP32R = mybir.dt.float32r
    BF16 = mybir.dt.bfloat16
    AF = mybir.ActivationFunctionType
    ALU = mybir.AluOpType
    AX = mybir.AxisListType

    nc = bacc.Bacc("TRN2", target_bir_lowering=False, debug=False)

    st_d = nc.dram_tensor("st", [BLOC, D, L], FP32R, kind="ExternalInput")
    qt_d = nc.dram_tensor("qt", [BLOC, D, L], FP32R, kind="ExternalInput")
    snb_d = nc.dram_tensor("snb", [BLOC, L, D], BF16, kind="ExternalInput")
    qnb_d = nc.dram_tensor("qnb", [BLOC, L, D], BF16, kind="ExternalInput")
    stb_d = nc.dram_tensor("stb", [BLOC, D, L], BF16, kind="ExternalInput")
    qtb_d = nc.dram_tensor("qtb", [BLOC, D, L], BF16, kind="ExternalInput")
    wr_d = nc.dram_tensor("wr", [NM, 128, KF, 128], BF16, kind="ExternalInput")
    wg_d = nc.dram_tensor("wg", [NM, 128, KF, 128], BF16, kind="ExternalInput")
    brt_d = nc.dram_tensor("brt", [128, NM], FP32, kind="ExternalInput")
    bgt_d = nc.dram_tensor("bgt", [128, NM], FP32, kind="ExternalInput")
    outs_d = nc.dram_tensor("outs", [BLOC, D, L], FP32, kind="ExternalOutput")
    outq_d = nc.dram_tensor("outq", [BLOC, D, L], FP32, kind="ExternalOutput")
    ident_d = nc.inline_tensor(np.eye(128, dtype=np.float32), name="identsrc")
    identb_d = nc.inline_tensor(
        np.eye(128, dtype=ml_dtypes.bfloat16), name="identbsrc")

    WAVES = [(0, 1, 2), (3, 4, 5), (6, 7)]

    with tile.TileContext(nc) as tc:
        with (
            tc.tile_pool(name="prog", bufs=1) as Pp,
            tc.tile_pool(name="qpool", bufs=1) as Pq,
        ):
            ident = Pp.tile([128, 128], FP32, tag="ident", name="ident")
            nc.sync.dma_start(ident[:], ident_d[:])
            identb = Pp.tile([128, 128], BF16, tag="identb", name="identb")
            nc.sync.dma_start(identb[:], identb_d[:])
            brt = Pp.tile([128, NM], FP32, tag="brt", name="brt")
            nc.sync.dma_start(brt[:], brt_d[:])
            bgt = Pp.tile([128, NM], FP32, tag="bgt", name="bgt")
            nc.sync.dma_start(bgt[:], bgt_d[:])

            def load_qtf(b):
                ts = []
                for k in range(NK):
                    t = Pq.tile([128, L], FP32R, tag="qtf", bufs=NK,
                                name=f"qtf{b}_{k}")
                    nc.sync.dma_start(t[:], qt_d[b, k * 128:(k + 1) * 128, :])
                    ts.append(t)
                return ts

            qtf = load_qtf(0)

            for b in range(BLOC):
                with tc.tile_pool(name=f"batch{b}", bufs=1) as Pb:
                    stats = {}
                    for nm_ in ("negm1", "d1", "lnd1", "l1n",
                                "negm2", "d2", "lnd2", "l2n"):
                        stats[nm_] = Pb.tile([128, NK], FP32, tag=nm_,
                                             name=f"{nm_}{b}")
                    stbt = [Pb.tile([128, L], BF16, tag="stb", bufs=NK,
                                    name=f"stb{b}_{k}") for k in range(NK)]
                    qtbt = [Pb.tile([128, L], BF16, tag="qtb", bufs=NK,
                                    name=f"qtb{b}_{k}") for k in range(NK)]
                    bT = []
                    cT = []

                    with tc.tile_pool(name=f"bc{b}", bufs=1) as Pc:
                        A = [Pc.tile([128, L], FP32, tag="A", bufs=NK,
                                     name=f"A{b}_{ms}") for ms in range(NK)]
                        p1t = Pc.tile([128, NK, L], BF16, tag="p1t",
                                      name=f"p1t{b}")
                        p2 = Pc.tile([128, NK, L], BF16, tag="p2",
                                     name=f"p2{b}")

                        def chain1(ms):
                            nc.vector.tensor_reduce(
                                stats["negm1"][:, ms:ms + 1], A[ms][:], AX.X,
                                ALU.max, negate=True)
                            esc = Pc.tile([128, L], BF16, tag="esc", bufs=1,
                                          name=f"esc{b}_{ms}")
                            nc.scalar.activation(
                                esc[:], A[ms][:], AF.Exp,
                                bias=stats["negm1"][:, ms:ms + 1],
                                accum_out=stats["d1"][:, ms:ms + 1])
                            nc.scalar.activation(
                                stats["lnd1"][:, ms:ms + 1],
                                stats["d1"][:, ms:ms + 1], AF.Ln)
                            nc.vector.tensor_sub(
                                stats["l1n"][:, ms:ms + 1],
                                stats["negm1"][:, ms:ms + 1],
                                stats["lnd1"][:, ms:ms + 1])
                            p1s = Pc.tile([128, L], BF16, tag="p1s", bufs=3,
                                          name=f"p1s{b}_{ms}")
                            nc.scalar.activation(
                                p1s[:], A[ms][:], AF.Exp,
                                bias=stats["l1n"][:, ms:ms + 1])
                            return p1s

                        p1s_tiles = {}

                        def t_p1(ms, pool):
                            strip = pool.tile([128, NK, 128], BF16,
                                              tag="strip", bufs=1,
                                              name=f"strp1{b}_{ms}")
                            for mt in range(NK):
                                nc.tensor.transpose(
                                    strip[:, mt, :],
                                    p1s_tiles[ms][:, mt * 128:(mt + 1) * 128],
                                    identb[:])
                            nc.vector.tensor_copy(
                                p1t[:, 0:NK, ms * 128:(ms + 1) * 128],
                                strip[:])

                        # ---------- S1: A = S Q^T + row softmax ----------
                        with tc.tile_pool(name=f"ps1_{b}", bufs=1,
                                          space="PSUM") as PS1:
                            for wi, wave in enumerate(WAVES):
                                pa = {}
                                for ms in wave:
                                    pa[ms] = [
                                        PS1.tile([128, 512], FP32, tag="pa",
                                                 bufs=7,
                                                 name=f"pa{b}_{ms}_{h}")
                                        for h in range(NH)]
                                for k in range(NK):
                                    for ms in wave:
                                        stf = Pc.tile(
                                            [128, 128], FP32R, tag="stf",
                                            bufs=16, name=f"stf{b}_{ms}_{k}")
                                        nc.sync.dma_start(
                                            stf[:],
                                            st_d[b, k * 128:(k + 1) * 128,
                                                 ms * 128:(ms + 1) * 128])
                                        for h in range(NH):
                                            nc.tensor.matmul(
                                                pa[ms][h][:], stf[:],
                                                qtf[k][:,
                                                       h * 512:(h + 1) * 512],
                                                start=(k == 0),
                                                stop=(k == NK - 1))
                                for ms in wave:
                                    for h in range(NH):
                                        nc.vector.tensor_copy(
                                            A[ms][:, h * 512:(h + 1) * 512],
                                            pa[ms][h][:])
                                if wi > 0:
                                    for ms in WAVES[wi - 1]:
                                        t_p1(ms, PS1)
                                for ms in wave:
                                    p1s_tiles[ms] = chain1(ms)

                        # ---------- B/C: A^T softmax + stage-2 ----------
                        for k in range(NK):
                            nc.sync.dma_start(
                                stbt[k][:], stb_d[b, k * 128:(k + 1) * 128, :])
                            nc.sync.dma_start(
                                qtbt[k][:], qtb_d[b, k * 128:(k + 1) * 128, :])

                        with tc.tile_pool(name=f"psbc_{b}", bufs=1,
                                          space="PSUM") as PSb:
                            at = {}

                            def at_mk(mt):
                                t = PSb.tile([128, L], FP32, tag="at", bufs=2,
                                             name=f"at{b}_{mt}")
                                for c in range(NK):
                                    nc.tensor.transpose(
                                        t[:, c * 128:(c + 1) * 128],
                                        A[c][:, mt * 128:(mt + 1) * 128],
                                        ident[:])
                                at[mt] = t

                            def chain2(mt):
                                nc.vector.tensor_reduce(
                                    stats["negm2"][:, mt:mt + 1], at[mt][:],
                                    AX.X, ALU.max, negate=True)
                                esc = Pc.tile([128, L], BF16, tag="esc",
                                              bufs=1, name=f"esc2{b}_{mt}")
                                nc.scalar.activation(
                                    esc[:], at[mt][:], AF.Exp,
                                    bias=stats["negm2"][:, mt:mt + 1],
                                    accum_out=stats["d2"][:, mt:mt + 1])
                                nc.scalar.activation(
                                    stats["lnd2"][:, mt:mt + 1],
                                    stats["d2"][:, mt:mt + 1], AF.Ln)
                                nc.vector.tensor_sub(
                                    stats["l2n"][:, mt:mt + 1],
                                    stats["negm2"][:, mt:mt + 1],
                                    stats["lnd2"][:, mt:mt + 1])
                                p2s = Pc.tile([128, L], BF16, tag="p2s",
                                              bufs=3, name=f"p2s{b}_{mt}")
                                nc.scalar.activation(
                                    p2s[:], at[mt][:], AF.Exp,
                                    bias=stats["l2n"][:, mt:mt + 1])
                                return p2s

                            p2s_tiles = {}

                            def t_p2(mt):
                                strip = PSb.tile([128, NK, 128], BF16,
                                                 tag="strip", bufs=1,
                                                 name=f"strp2{b}_{mt}")
                                for ms in range(NK):
                                    nc.tensor.transpose(
                                        strip[:, ms, :],
                                        p2s_tiles[mt][:,
                                                      ms * 128:(ms + 1) * 128],
                                        identb[:])
                                nc.vector.tensor_copy(
                                    p2[:, 0:NK, mt * 128:(mt + 1) * 128],
                                    strip[:])

                            # warm-up: first two at tiles + S1 tail
                            # transposes (their ACT chains need the gap)
                            at_mk(0)
                            t_p1(WAVES[-1][0], PSb)
                            at_mk(1)
                            t_p1(WAVES[-1][1], PSb)
                            p2s_tiles[0] = chain2(0)
                            p2s_tiles[1] = chain2(1)

                            def bmm(md, nat_d, pmov, out_list, tagc):
                                pb = [PSb.tile([128, 512], FP32, tag="pb",
                                               bufs=3,
                                               name=f"pb{b}_{tagc}{md}_{h}")
                                      for h in range(NH)]
                                for kt in range(NK):
                                    ch = Pc.tile(
                                        [128, 128], BF16, tag=f"{tagc}chunk",
                                        bufs=16, name=f"{tagc}ch{b}_{md}_{kt}")
                                    nc.sync.dma_start(
                                        ch[:],
                                        nat_d[b, kt * 128:(kt + 1) * 128,
                                              md * 128:(md + 1) * 128])
                                    for h in range(NH):
                                        nc.tensor.matmul(
                                            pb[h][:], ch[:],
                                            pmov[:, kt, h * 512:(h + 1) * 512],
                                            start=(kt == 0),
                                            stop=(kt == NK - 1))
                                ot = Pb.tile([128, L], BF16, tag=f"{tagc}T",
                                             bufs=NM, name=f"{tagc}T{b}_{md}")
                                for h in range(NH):
                                    nc.vector.tensor_copy(
                                        ot[:, h * 512:(h + 1) * 512], pb[h][:])
                                out_list.append(ot)

                            for md in range(NM):
                                bmm(md, qnb_d, p1t, bT, "b")
                                if md >= 1:
                                    t_p2(md - 1)
                                mt = md + 2
                                if mt < NK:
                                    at_mk(mt)
                                    p2s_tiles[mt] = chain2(mt)
                            t_p2(NK - 2)
                            t_p2(NK - 1)
                            for md in range(NM):
                                bmm(md, snb_d, p2, cT, "c")

                    # ---------- H: heuristic ----------
                    with (
                        tc.tile_pool(name=f"heur{b}", bufs=1) as Ph,
                        tc.tile_pool(name=f"psH{b}", bufs=7,
                                     space="PSUM") as PSh,
                    ):
                        xys = []
                        xyq = []
                        for k in range(NK):
                            t1 = Ph.tile([128, L], BF16, tag="xys", bufs=NK,
                                         name=f"xys{b}_{k}")
                            nc.vector.tensor_mul(t1[:], stbt[k][:], bT[k][:])
                            xys.append(t1)
                            t2 = Ph.tile([128, L], BF16, tag="xyq", bufs=NK,
                                         name=f"xyq{b}_{k}")
                            nc.vector.tensor_mul(t2[:], qtbt[k][:], cT[k][:])
                            xyq.append(t2)

                        if b + 1 < BLOC:
                            qtf = load_qtf(b + 1)

                        for m in range(NM):
                            wrt = Ph.tile([128, KF, 128], BF16, tag="wr",
                                          bufs=2, name=f"wrt{b}_{m}")
                            nc.sync.dma_start(wrt[:], wr_d[m])
                            wgt = Ph.tile([128, KF, 128], BF16, tag="wg",
                                          bufs=2, name=f"wgt{b}_{m}")
                            nc.sync.dma_start(wgt[:], wg_d[m])
                            for xt, blocks, outd in (
                                (stbt, (stbt, bT, xys), outs_d),
                                (qtbt, (qtbt, cT, xyq), outq_d),
                            ):
                                tag = "s" if outd is outs_d else "q"
                                pr = [PSh.tile([128, 512], FP32, tag="rg",
                                               bufs=7,
                                               name=f"pr{b}_{m}{tag}{h}")
                                      for h in range(NH)]
                                pg = [PSh.tile([128, 512], FP32, tag="rg",
                                               bufs=7,
                                               name=f"pg{b}_{m}{tag}{h}")
                                      for h in range(NH)]
                                for kf in range(KF):
                                    rhs = blocks[kf // NK][kf % NK]
                                    for h in range(NH):
                                        nc.tensor.matmul(
                                            pr[h][:], wrt[:, kf, :],
                                            rhs[:, h * 512:(h + 1) * 512],
                                            start=(kf == 0),
                                            stop=(kf == KF - 1))
                                    for h in range(NH):
                                        nc.tensor.matmul(
                                            pg[h][:], wgt[:, kf, :],
                                            rhs[:, h * 512:(h + 1) * 512],
                                            start=(kf == 0),
                                            stop=(kf == KF - 1))
                                r_sb = Ph.tile([128, L], BF16, tag="rsb",
                                               bufs=2, name=f"rsb{b}_{m}{tag}")
                                g_sb = Ph.tile([128, L], BF16, tag="gsb",
                                               bufs=2, name=f"gsb{b}_{m}{tag}")
                                for h in range(NH):
                                    nc.scalar.activation(
                                        r_sb[:, h * 512:(h + 1) * 512],
                                        pr[h][:], AF.Gelu_apprx_tanh,
                                        bias=brt[:, m:m + 1])
                                for h in range(NH):
                                    nc.scalar.activation(
                                        g_sb[:, h * 512:(h + 1) * 512],
                                        pg[h][:], AF.Sigmoid,
                                        bias=bgt[:, m:m + 1])
                                t1 = Ph.tile([128, L], FP32, tag="t1", bufs=2,
                                             name=f"t1{b}_{m}{tag}")
                                nc.vector.tensor_sub(t1[:], r_sb[:], xt[m][:])
                                t2 = Ph.tile([128, L], FP32, tag="t2", bufs=2,
                                             name=f"t2{b}_{m}{tag}")
                                nc.gpsimd.tensor_mul(t2[:], g_sb[:], t1[:])
                                osb = Ph.tile([128, L], FP32, tag="osb",
                                              bufs=2, name=f"osb{b}_{m}{tag}")
                                nc.vector.tensor_add(osb[:], t2[:], xt[m][:])
                                nc.sync.dma_start(
                                    outd[b, m * 128:(m + 1) * 128, :], osb[:])

    nc.compile()
    return nc


def _get_nc():
    global _nc_cache
    if _nc_cache is None:
        _nc_cache = _build()
    return _nc_cache


def _prep_inputs(s, q, w_r, b_r, w_g, b_g):
    bf = ml_dtypes.bfloat16
    s = np.ascontiguousarray(np.asarray(s, dtype=np.float32))
    q = np.ascontiguousarray(np.asarray(q, dtype=np.float32))
    w_r = np.asarray(w_r, dtype=np.float32)
    w_g = np.asarray(w_g, dtype=np.float32)
    b_r = np.asarray(b_r, dtype=np.float32)
    b_g = np.asarray(b_g, dtype=np.float32)

    st = np.ascontiguousarray(s.transpose(0, 2, 1))
    qt = np.ascontiguousarray(q.transpose(0, 2, 1))
    snb = s.astype(bf)
    qnb = q.astype(bf)
    stb = st.astype(bf)
    qtb = qt.astype(bf)

    def pack_w(w):
        W1, W2, W3, W4 = (w[:, i * D:(i + 1) * D] for i in range(4))
        eff = np.concatenate([W1 + W4, W2 - W4, W3], axis=1)  # [D, 3D]
        wt = eff.T  # [3D, D]
        pk = wt.reshape(KF, 128, NM, 128).transpose(2, 1, 0, 3)  # [m, f, k, o]
        return np.ascontiguousarray(pk).astype(bf)

    wr_pack = pack_w(w_r)
    wg_pack = pack_w(w_g)
    brt = np.ascontiguousarray(b_r.reshape(NM, 128).T)
    bgt = np.ascontiguousarray(b_g.reshape(NM, 128).T)

    in_maps = []
    for c in range(NCORES):
        sl = slice(BLOC * c, BLOC * (c + 1))
        in_maps.append({
            "st": st[sl], "qt": qt[sl],
            "snb": snb[sl], "qnb": qnb[sl],
            "stb": stb[sl], "qtb": qtb[sl],
            "wr": wr_pack, "wg": wg_pack,
            "brt": brt, "bgt": bgt,
        })
    return in_maps


def run(inputs, trace=False, tmpdir=None):
    """Execute on 8 NeuronCores; returns ((s_tilde, q_tilde), BassKernelResults)."""
    from concourse.bass_utils import run_bass_kernel_spmd

    in_maps = _prep_inputs(
        inputs["s"], inputs["q"], inputs["w_r"], inputs["b_r"],
        inputs["w_g"], inputs["b_g"])
    nc = _get_nc()
    res = run_bass_kernel_spmd(nc, in_maps, list(range(NCORES)), trace=trace,
                               tmpdir=tmpdir)
    s_t = np.empty((B, L, D), np.float32)
    q_t = np.empty((B, L, D), np.float32)
    for c in range(NCORES):
        sl = slice(BLOC * c, BLOC * (c + 1))
        s_t[sl] = res.results[c]["outs"].transpose(0, 2, 1)
        q_t[sl] = res.results[c]["outq"].transpose(0, 2, 1)
    return (s_t, q_t), res


def kernel(s, q, w_r, b_r, w_g, b_g, s_mask=None, q_mask=None):
    # s_mask / q_mask are all-ones in this problem; the additive mask term
    # (1 - m1*m2) * NEG_INF is identically zero, so they are unused.
    out, _ = run({"s": s, "q": q, "w_r": w_r, "b_r": b_r,
                  "w_g": w_g, "b_g": b_g})
    return out


# revision 14
# speedup vs baseline: 1.1806x; 1.0316x over previous
"""Trainium2 Bass kernel for nn_Attention_65223373357517.

Computes, for s,q [B=16, L=1024, D=1024] (D = 2H, H=512):
    a  = einsum('bsd,btd->bst', s, q)
    b  = softmax(a, -1) @ q
    c  = softmax(a^T, -1) @ s
    s~ = heuristic(s, b);  q~ = heuristic(q, c)
with heuristic(x, y) = g*r + (1-g)*x,
    r = gelu_tanh([x, y, x*y, x-y] @ w_r.T + b_r)
    g = sigmoid ([x, y, x*y, x-y] @ w_g.T + b_g)

Strategy: pure data-parallel over batch (2 examples per NeuronCore, 8 cores,
no collectives). Host folds the (x-y) block into the x/y weight blocks
(W1+W4, W2-W4, W3), transposes activations so every on-chip matmul is in
its natural layout, and transposes outputs back.  Masks are all-ones in
this problem configuration (additive mask term is identically zero), so
they do not enter the computation.

On-chip per batch (v2 schedule — PE kept saturated):
  S1:  A = S Q^T via f32r matmuls.  qt rows resident (prefetched during
       the previous batch's heuristic); st streamed as [128,128]
       stationary chunks, each reused for both 512-halves.  ms processed
       in waves of 3 (PSUM bound), k inner so compute chases DMA.
       Per A tile: row max m1, d1 = sum exp(A-m1) (fused ACT accum),
       l1 = m1 + ln d1 folded into the ACT bias so P1 = exp(A - l1) is
       the *normalized* softmax in A layout (bf16, per-partition bias —
       no cross-partition broadcast needed).  PE bf16-transposes P1 into
       p1t strips, issued one wave late so the ACT chain stays hidden.
  B/C: A^T tiles via PE fp32 transposes (PSUM); same per-tile stat chain
       gives P2^T = exp(A^T - l2) bf16, transposed back into p2 strips.
       b^T = Q_nat-chunk-contracted matmuls with rhs p1t; c^T likewise
       with lhsT = S_nat chunks, rhs p2.  at/p2 transposes interleave
       with the b^T matmul stream so PE never waits on stat chains.
  H:   heuristic per 128-row output strip: 24 K-chunk bf16 matmuls each
       for r and g over blocks [x^T, y^T, (x*y)^T]; gelu/sigmoid read
       PSUM with per-partition bias; epilogue out = x + g*(r - x);
       stream out s~^T / q~^T.  Next batch's qt prefetches here.
"""

import numpy as np
import ml_dtypes

B, L, D = 16, 1024, 1024
NCORES = 8
BLOC = B // NCORES          # batches per core
NK = D // 128               # contraction chunks (8)
NM = D // 128               # output-row chunks (8)
KF = 3 * D // 128           # folded heuristic contraction chunks (24)
NH = 2                      # 512-wide halves of a 1024 free dim

_nc_cache = None


def _build():
    import concourse.tile as tile
    from concourse import bacc, mybir

    FP32 = mybir.dt.float32
    FP32R = mybir.dt.float32r
    BF16 = mybir.dt.bfloat16
    AF = mybir.ActivationFunctionType
    ALU = mybir.AluOpType
    AX = mybir.AxisListType

    nc = bacc.Bacc("TRN2", target_bir_lowering=False, debug=False)

    st_d = nc.dram_tensor("st", [BLOC, D, L], FP32R, kind="ExternalInput")
    qt_d = nc.dram_tensor("qt", [BLOC, D, L], FP32R, kind="ExternalInput")
    snb_d = nc.dram_tensor("snb", [BLOC, L, D], BF16, kind="ExternalInput")
    qnb_d = nc.dram_tensor("qnb", [BLOC, L, D], BF16, kind="ExternalInput")
    stb_d = nc.dram_tensor("stb", [BLOC, D, L], BF16, kind="ExternalInput")
    qtb_d = nc.dram_tensor("qtb", [BLOC, D, L], BF16, kind="ExternalInput")
    wr_d = nc.dram_tensor("wr", [NM, 128, KF, 128], BF16, kind="ExternalInput")
    wg_d = nc.dram_tensor("wg", [NM, 128, KF, 128], BF16, kind="ExternalInput")
    brt_d = nc.dram_tensor("brt", [128, NM], FP32, kind="ExternalInput")
    bgt_d = nc.dram_tensor("bgt", [128, NM], FP32, kind="ExternalInput")
    outs_d = nc.dram_tensor("outs", [BLOC, D, L], FP32, kind="ExternalOutput")
    outq_d = nc.dram_tensor("outq", [BLOC, D, L], FP32, kind="ExternalOutput")
    ident_d = nc.inline_tensor(np.eye(128, dtype=np.float32), name="identsrc")
    identb_d = nc.inline_tensor(
        np.eye(128, dtype=ml_dtypes.bfloat16), name="identbsrc")

    WAVES = [(0, 1, 2), (3, 4, 5), (6, 7)]

    with tile.TileContext(nc) as tc:
        # consumption order of (k, ms) stationary chunks across S1 waves
        ORDER = [(k, ms) for wave in WAVES for k in range(NK) for ms in wave]
        PRE = 16  # stf ring depth = cross-batch prefetch count

        with (
            tc.tile_pool(name="prog", bufs=1) as Pp,
            tc.tile_pool(name="qpool", bufs=1) as Pq,
            tc.tile_pool(name="stpool", bufs=1) as Pst,
        ):
            ident = Pp.tile([128, 128], FP32, tag="ident", name="ident")
            nc.sync.dma_start(ident[:], ident_d[:])
            identb = Pp.tile([128, 128], BF16, tag="identb", name="identb")
            nc.sync.dma_start(identb[:], identb_d[:])
            brt = Pp.tile([128, NM], FP32, tag="brt", name="brt")
            nc.sync.dma_start(brt[:], brt_d[:])
            bgt = Pp.tile([128, NM], FP32, tag="bgt", name="bgt")
            nc.sync.dma_start(bgt[:], bgt_d[:])

            def qtf_dma(b, k):
                t = Pq.tile([128, L], FP32R, tag="qtf", bufs=NK,
                            name=f"qtf{b}_{k}")
                nc.sync.dma_start(t[:], qt_d[b, k * 128:(k + 1) * 128, :])
                return t

            def stf_dma(b, k, ms):
                t = Pst.tile([128, 128], FP32R, tag="stf", bufs=PRE,
                             name=f"stf{b}_{k}_{ms}")
                nc.sync.dma_start(
                    t[:], st_d[b, k * 128:(k + 1) * 128,
                               ms * 128:(ms + 1) * 128])
                return t

            # batch 0 inputs: interleave qt rows with the first stationary
            # chunks so wave-0 matmuls chase DMA arrival
            qtf = []
            stf_pre = {}
            pi = 0
            for k in range(NK):
                qtf.append(qtf_dma(0, k))
                while pi < PRE and ORDER[pi][0] <= k + 1:
                    stf_pre[ORDER[pi]] = stf_dma(0, *ORDER[pi])
                    pi += 1

            for b in range(BLOC):
                with tc.tile_pool(name=f"batch{b}", bufs=1) as Pb:
                    stats = {}
                    for nm_ in ("negm1", "d1", "lnd1", "l1n",
                                "negm2", "d2", "lnd2", "l2n"):
                        stats[nm_] = Pb.tile([128, NK], FP32, tag=nm_,
                                             name=f"{nm_}{b}")
                    stbt = [Pb.tile([128, L], BF16, tag="stb", bufs=NK,
                                    name=f"stb{b}_{k}") for k in range(NK)]
                    qtbt = [Pb.tile([128, L], BF16, tag="qtb", bufs=NK,
                                    name=f"qtb{b}_{k}") for k in range(NK)]
                    bT = []
                    cT = []

                    with tc.tile_pool(name=f"bc{b}", bufs=1) as Pc:
                        A = [Pc.tile([128, L], FP32, tag="A", bufs=NK,
                                     name=f"A{b}_{ms}") for ms in range(NK)]
                        p1t = Pc.tile([128, NK, L], BF16, tag="p1t",
                                      name=f"p1t{b}")
                        p2 = Pc.tile([128, NK, L], BF16, tag="p2",
                                     name=f"p2{b}")

                        def chain1(ms):
                            nc.vector.tensor_reduce(
                                stats["negm1"][:, ms:ms + 1], A[ms][:], AX.X,
                                ALU.max, negate=True)
                            esc = Pc.tile([128, L], BF16, tag="esc", bufs=1,
                                          name=f"esc{b}_{ms}")
                            nc.scalar.activation(
                                esc[:], A[ms][:], AF.Exp,
                                bias=stats["negm1"][:, ms:ms + 1],
                                accum_out=stats["d1"][:, ms:ms + 1])
                            nc.scalar.activation(
                                stats["lnd1"][:, ms:ms + 1],
                                stats["d1"][:, ms:ms + 1], AF.Ln)
                            nc.vector.tensor_sub(
                                stats["l1n"][:, ms:ms + 1],
                                stats["negm1"][:, ms:ms + 1],
                                stats["lnd1"][:, ms:ms + 1])
                            p1s = Pc.tile([128, L], BF16, tag="p1s", bufs=3,
                                          name=f"p1s{b}_{ms}")
                            nc.scalar.activation(
                                p1s[:], A[ms][:], AF.Exp,
                                bias=stats["l1n"][:, ms:ms + 1])
                            return p1s

                        p1s_tiles = {}

                        def t_p1(ms, pool):
                            strip = pool.tile([128, NK, 128], BF16,
                                              tag="strip", bufs=1,
                                              name=f"strp1{b}_{ms}")
                            for mt in range(NK):
                                nc.tensor.transpose(
                                    strip[:, mt, :],
                                    p1s_tiles[ms][:, mt * 128:(mt + 1) * 128],
                                    identb[:])
                            nc.vector.tensor_copy(
                                p1t[:, 0:NK, ms * 128:(ms + 1) * 128],
                                strip[:])

                        # ---------- S1: A = S Q^T + row softmax ----------
                        with tc.tile_pool(name=f"ps1_{b}", bufs=1,
                                          space="PSUM") as PS1:
                            for wi, wave in enumerate(WAVES):
                                pa = {}
                                for ms in wave:
                                    pa[ms] = [
                                        PS1.tile([128, 512], FP32, tag="pa",
                                                 bufs=7,
                                                 name=f"pa{b}_{ms}_{h}")
                                        for h in range(NH)]
                                for k in range(NK):
                                    for ms in wave:
                                        stf = stf_pre.pop(
                                            (k, ms), None) or stf_dma(b, k, ms)
                                        for h in range(NH):
                                            nc.tensor.matmul(
                                                pa[ms][h][:], stf[:],
                                                qtf[k][:,
                                                       h * 512:(h + 1) * 512],
                                                start=(k == 0),
                                                stop=(k == NK - 1))
                                for ms in wave:
                                    for h in range(NH):
                                        nc.vector.tensor_copy(
                                            A[ms][:, h * 512:(h + 1) * 512],
                                            pa[ms][h][:])
                                if wi > 0:
                                    for ms in WAVES[wi - 1]:
                                        t_p1(ms, PS1)
                                for ms in wave:
                                    p1s_tiles[ms] = chain1(ms)

                        # ---------- B/C: A^T softmax + stage-2 ----------
                        with tc.tile_pool(name=f"psbc_{b}", bufs=1,
                                          space="PSUM") as PSb:
                            at = {}

                            def at_mk(mt):
                                t = PSb.tile([128, L], FP32, tag="at", bufs=2,
                                             name=f"at{b}_{mt}")
                                for c in range(NK):
                                    nc.tensor.transpose(
                                        t[:, c * 128:(c + 1) * 128],
                                        A[c][:, mt * 128:(mt + 1) * 128],
                                        ident[:])
                                at[mt] = t

                            def chain2(mt):
                                nc.vector.tensor_reduce(
                                    stats["negm2"][:, mt:mt + 1], at[mt][:],
                                    AX.X, ALU.max, negate=True)
                                esc = Pc.tile([128, L], BF16, tag="esc",
                                              bufs=1, name=f"esc2{b}_{mt}")
                                nc.scalar.activation(
                                    esc[:], at[mt][:], AF.Exp,
                                    bias=stats["negm2"][:, mt:mt + 1],
                                    accum_out=stats["d2"][:, mt:mt + 1])
                                nc.scalar.activation(
                                    stats["lnd2"][:, mt:mt + 1],
                                    stats["d2"][:, mt:mt + 1], AF.Ln)
                                nc.vector.tensor_sub(
                                    stats["l2n"][:, mt:mt + 1],
                                    stats["negm2"][:, mt:mt + 1],
                                    stats["lnd2"][:, mt:mt + 1])
                                p2s = Pc.tile([128, L], BF16, tag="p2s",
                                              bufs=3, name=f"p2s{b}_{mt}")
                                nc.scalar.activation(
                                    p2s[:], at[mt][:], AF.Exp,
                                    bias=stats["l2n"][:, mt:mt + 1])
                                return p2s

                            p2s_tiles = {}

                            def t_p2(mt):
                                strip = PSb.tile([128, NK, 128], BF16,
                                                 tag="strip", bufs=1,
                                                 name=f"strp2{b}_{mt}")
                                for ms in range(NK):
                                    nc.tensor.transpose(
                                        strip[:, ms, :],
                                        p2s_tiles[mt][:,
                                                      ms * 128:(ms + 1) * 128],
                                        identb[:])
                                nc.vector.tensor_copy(
                                    p2[:, 0:NK, mt * 128:(mt + 1) * 128],
                                    strip[:])

                            # warm-up: first two at tiles + S1 tail
                            # transposes (their ACT chains need the gap)
                            at_mk(0)
                            t_p1(WAVES[-1][0], PSb)
                            at_mk(1)
                            t_p1(WAVES[-1][1], PSb)
                            p2s_tiles[0] = chain2(0)
                            p2s_tiles[1] = chain2(1)

                            def bmm(md, nat_d, pmov, out_list, tagc):
                                pb = [PSb.tile([128, 512], FP32, tag="pb",
                                               bufs=3,
                                               name=f"pb{b}_{tagc}{md}_{h}")
                                      for h in range(NH)]
                                for kt in range(NK):
                                    ch = Pc.tile(
                                        [128, 128], BF16, tag=f"{tagc}chunk",
                                        bufs=16, name=f"{tagc}ch{b}_{md}_{kt}")
                                    nc.sync.dma_start(
                                        ch[:],
                                        nat_d[b, kt * 128:(kt + 1) * 128,
                                              md * 128:(md + 1) * 128])
                                    for h in range(NH):
                                        nc.tensor.matmul(
                                            pb[h][:], ch[:],
                                            pmov[:, kt, h * 512:(h + 1) * 512],
                                            start=(kt == 0),
                                            stop=(kt == NK - 1))
                                ot = Pb.tile([128, L], BF16, tag=f"{tagc}T",
                                             bufs=NM, name=f"{tagc}T{b}_{md}")
                                for h in range(NH):
                                    nc.vector.tensor_copy(
                                        ot[:, h * 512:(h + 1) * 512], pb[h][:])
                                out_list.append(ot)

                            w0 = {}
                            for md in range(NM):
                                bmm(md, qnb_d, p1t, bT, "b")
                                if md == 1:
                                    # prefetch first heuristic weight pair
                                    for wn, wd in (("wr", wr_d), ("wg", wg_d)):
                                        t = Pb.tile([128, KF, 128], BF16,
                                                    tag=f"{wn}0",
                                                    name=f"{wn}0_{b}")
                                        nc.sync.dma_start(t[:], wd[0])
                                        w0[wn] = t
                                if md >= 1:
                                    t_p2(md - 1)
                                mt = md + 2
                                if mt < NK:
                                    at_mk(mt)
                                    p2s_tiles[mt] = chain2(mt)
                            t_p2(NK - 2)
                            t_p2(NK - 1)
                            for k in range(NK):
                                nc.sync.dma_start(
                                    stbt[k][:],
                                    stb_d[b, k * 128:(k + 1) * 128, :])
                                nc.sync.dma_start(
                                    qtbt[k][:],
                                    qtb_d[b, k * 128:(k + 1) * 128, :])
                            for md in range(NM):
                                bmm(md, snb_d, p2, cT, "c")

                    # ---------- H: heuristic ----------
                    with (
                        tc.tile_pool(name=f"heur{b}", bufs=1) as Ph,
                        tc.tile_pool(name=f"psH{b}", bufs=7,
                                     space="PSUM") as PSh,
                    ):
                        xys = []
                        xyq = []
                        for k in range(NK):
                            t1 = Ph.tile([128, L], BF16, tag="xys", bufs=NK,
                                         name=f"xys{b}_{k}")
                            nc.vector.tensor_mul(t1[:], stbt[k][:], bT[k][:])
                            xys.append(t1)
                            t2 = Ph.tile([128, L], BF16, tag="xyq", bufs=NK,
                                         name=f"xyq{b}_{k}")
                            nc.vector.tensor_mul(t2[:], qtbt[k][:], cT[k][:])
                            xyq.append(t2)

                        if b + 1 < BLOC:
                            qtf = [qtf_dma(b + 1, k) for k in range(NK)]
                            for i in range(PRE):
                                stf_pre[ORDER[i]] = stf_dma(b + 1, *ORDER[i])

                        for m in range(NM):
                            if m == 0:
                                wrt, wgt = w0["wr"], w0["wg"]
                            else:
                                wrt = Ph.tile([128, KF, 128], BF16, tag="wr",
                                              bufs=2, name=f"wrt{b}_{m}")
                                nc.sync.dma_start(wrt[:], wr_d[m])
                                wgt = Ph.tile([128, KF, 128], BF16, tag="wg",
                                              bufs=2, name=f"wgt{b}_{m}")
                                nc.sync.dma_start(wgt[:], wg_d[m])
                            for xt, blocks, outd in (
                                (stbt, (stbt, bT, xys), outs_d),
                                (qtbt, (qtbt, cT, xyq), outq_d),
                            ):
                                tag = "s" if outd is outs_d else "q"
                                pr = [PSh.tile([128, 512], FP32, tag="rg",
                                               bufs=7,
                                               name=f"pr{b}_{m}{tag}{h}")
                                      for h in range(NH)]
                                pg = [PSh.tile([128, 512], FP32, tag="rg",
                                               bufs=7,
                                               name=f"pg{b}_{m}{tag}{h}")
                                      for h in range(NH)]
                                for kf in range(KF):
                                    rhs = blocks[kf // NK][kf % NK]
                                    for h in range(NH):
                                        nc.tensor.matmul(
                                            pr[h][:], wrt[:, kf, :],
                                            rhs[:, h * 512:(h + 1) * 512],
                                            start=(kf == 0),
                                            stop=(kf == KF - 1))
                                    for h in range(NH):
                                        nc.tensor.matmul(
                                            pg[h][:], wgt[:, kf, :],
                                            rhs[:, h * 512:(h + 1) * 512],
                                            start=(kf == 0),
                                            stop=(kf == KF - 1))
                                r_sb = Ph.tile([128, L], BF16, tag="rsb",
                                               bufs=2, name=f"rsb{b}_{m}{tag}")
                                g_sb = Ph.tile([128, L], BF16, tag="gsb",
                                               bufs=2, name=f"gsb{b}_{m}{tag}")
                                for h in range(NH):
                                    nc.scalar.activation(
                                        r_sb[:, h * 512:(h + 1) * 512],
                                        pr[h][:], AF.Gelu_apprx_tanh,
                                        bias=brt[:, m:m + 1])
                                for h in range(NH):
                                    nc.scalar.activation(
                                        g_sb[:, h * 512:(h + 1) * 512],
                                        pg[h][:], AF.Sigmoid,
                                        bias=bgt[:, m:m + 1])
                                t1 = Ph.tile([128, L], FP32, tag="t1", bufs=2,
                                             name=f"t1{b}_{m}{tag}")
                                nc.vector.tensor_sub(t1[:], r_sb[:], xt[m][:])
                                t2 = Ph.tile([128, L], FP32, tag="t2", bufs=2,
                                             name=f"t2{b}_{m}{tag}")
                                nc.vector.tensor_mul(t2[:], g_sb[:], t1[:])
                                osb = Ph.tile([128, L], FP32, tag="osb",
                                              bufs=2, name=f"osb{b}_{m}{tag}")
                                nc.vector.tensor_add(osb[:], t2[:], xt[m][:])
                                nc.sync.dma_start(
                                    outd[b, m * 128:(m + 1) * 128, :], osb[:])

    nc.compile()
    return nc


def _get_nc():
    global _nc_cache
    if _nc_cache is None:
        _nc_cache = _build()
    return _nc_cache


def _prep_inputs(s, q, w_r, b_r, w_g, b_g):
    bf = ml_dtypes.bfloat16
    s = np.ascontiguousarray(np.asarray(s, dtype=np.float32))
    q = np.ascontiguousarray(np.asarray(q, dtype=np.float32))
    w_r = np.asarray(w_r, dtype=np.float32)
    w_g = np.asarray(w_g, dtype=np.float32)
    b_r = np.asarray(b_r, dtype=np.float32)
    b_g = np.asarray(b_g, dtype=np.float32)

    st = np.ascontiguousarray(s.transpose(0, 2, 1))
    qt = np.ascontiguousarray(q.transpose(0, 2, 1))
    snb = s.astype(bf)
    qnb = q.astype(bf)
    stb = st.astype(bf)
    qtb = qt.astype(bf)

    def pack_w(w):
        W1, W2, W3, W4 = (w[:, i * D:(i + 1) * D] for i in range(4))
        eff = np.concatenate([W1 + W4, W2 - W4, W3], axis=1)  # [D, 3D]
        wt = eff.T  # [3D, D]
        pk = wt.reshape(KF, 128, NM, 128).transpose(2, 1, 0, 3)  # [m, f, k, o]
        return np.ascontiguousarray(pk).astype(bf)

    wr_pack = pack_w(w_r)
    wg_pack = pack_w(w_g)
    brt = np.ascontiguousarray(b_r.reshape(NM, 128).T)
    bgt = np.ascontiguousarray(b_g.reshape(NM, 128).T)

    in_maps = []
    for c in range(NCORES):
        sl = slice(BLOC * c, BLOC * (c + 1))
        in_maps.append({
            "st": st[sl], "qt": qt[sl],
            "snb": snb[sl], "qnb": qnb[sl],
            "stb": stb[sl], "qtb": qtb[sl],
            "wr": wr_pack, "wg": wg_pack,
            "brt": brt, "bgt": bgt,
        })
    return in_maps


def run(inputs, trace=False, tmpdir=None):
    """Execute on 8 NeuronCores; returns ((s_tilde, q_tilde), BassKernelResults)."""
    from concourse.bass_utils import run_bass_kernel_spmd

    in_maps = _prep_inputs(
        inputs["s"], inputs["q"], inputs["w_r"], inputs["b_r"],
        inputs["w_g"], inputs["b_g"])
    nc = _get_nc()
    res = run_bass_kernel_spmd(nc, in_maps, list(range(NCORES)), trace=trace,
                               tmpdir=tmpdir)
    s_t = np.empty((B, L, D), np.float32)
    q_t = np.empty((B, L, D), np.float32)
    for c in range(NCORES):
        sl = slice(BLOC * c, BLOC * (c + 1))
        s_t[sl] = res.results[c]["outs"].transpose(0, 2, 1)
        q_t[sl] = res.results[c]["outq"].transpose(0, 2, 1)
    return (s_t, q_t), res


def kernel(s, q, w_r, b_r, w_g, b_g, s_mask=None, q_mask=None):
    # s_mask / q_mask are all-ones in this problem; the additive mask term
    # (1 - m1*m2) * NEG_INF is identically zero, so they are unused.
    out, _ = run({"s": s, "q": q, "w_r": w_r, "b_r": b_r,
                  "w_g": w_g, "b_g": b_g})
    return out


# revision 24
# speedup vs baseline: 1.3148x; 1.1137x over previous
"""Trainium2 Bass kernel for nn_Attention_65223373357517.

Computes, for s,q [B=16, L=1024, D=1024] (D = 2H, H=512):
    a  = einsum('bsd,btd->bst', s, q)
    b  = softmax(a, -1) @ q
    c  = softmax(a^T, -1) @ s
    s~ = heuristic(s, b);  q~ = heuristic(q, c)
with heuristic(x, y) = g*r + (1-g)*x,
    r = gelu_tanh([x, y, x*y, x-y] @ w_r.T + b_r)
    g = sigmoid ([x, y, x*y, x-y] @ w_g.T + b_g)

Strategy: pure data-parallel over batch (2 examples per NeuronCore, 8 cores,
no collectives). Host folds the (x-y) block into the x/y weight blocks
(W1+W4, W2-W4, W3), transposes activations so every on-chip matmul is in
its natural layout, and transposes outputs back.  Masks are all-ones in
this problem configuration (additive mask term is identically zero), so
they do not enter the computation.

On-chip per batch (v2 schedule — PE kept saturated):
  S1:  A = S Q^T via f32r matmuls.  qt rows resident (prefetched during
       the previous batch's heuristic); st streamed as [128,128]
       stationary chunks, each reused for both 512-halves.  ms processed
       in waves of 3 (PSUM bound), k inner so compute chases DMA.
       Per A tile: row max m1, d1 = sum exp(A-m1) (fused ACT accum),
       l1 = m1 + ln d1 folded into the ACT bias so P1 = exp(A - l1) is
       the *normalized* softmax in A layout (bf16, per-partition bias —
       no cross-partition broadcast needed).  PE bf16-transposes P1 into
       p1t strips, issued one wave late so the ACT chain stays hidden.
  B/C: A^T tiles via PE fp32 transposes (PSUM); same per-tile stat chain
       gives P2^T = exp(A^T - l2) bf16, transposed back into p2 strips.
       b^T = Q_nat-chunk-contracted matmuls with rhs p1t; c^T likewise
       with lhsT = S_nat chunks, rhs p2.  at/p2 transposes interleave
       with the b^T matmul stream so PE never waits on stat chains.
  H:   heuristic per 128-row output strip: 24 K-chunk bf16 matmuls each
       for r and g over blocks [x^T, y^T, (x*y)^T]; gelu/sigmoid read
       PSUM with per-partition bias; epilogue out = x + g*(r - x);
       stream out s~^T / q~^T.  Next batch's qt prefetches here.
"""

import numpy as np
import ml_dtypes

B, L, D = 16, 1024, 1024
NCORES = 8
BLOC = B // NCORES          # batches per core
NK = D // 128               # contraction chunks (8)
NM = D // 128               # output-row chunks (8)
KF = 3 * D // 128           # folded heuristic contraction chunks (24)
NH = 2                      # 512-wide halves of a 1024 free dim

_nc_cache = None


def _build():
    import concourse.tile as tile
    from concourse import bacc, mybir

    FP32 = mybir.dt.float32
    FP32R = mybir.dt.float32r
    BF16 = mybir.dt.bfloat16
    AF = mybir.ActivationFunctionType
    ALU = mybir.AluOpType
    AX = mybir.AxisListType

    nc = bacc.Bacc("TRN2", target_bir_lowering=False, debug=False)

    st_d = nc.dram_tensor("st", [BLOC, D, L], FP32R, kind="ExternalInput")
    qt_d = nc.dram_tensor("qt", [BLOC, D, L], FP32R, kind="ExternalInput")
    snb_d = nc.dram_tensor("snb", [BLOC, L, D], BF16, kind="ExternalInput")
    qnb_d = nc.dram_tensor("qnb", [BLOC, L, D], BF16, kind="ExternalInput")
    stb_d = nc.dram_tensor("stb", [BLOC, D, L], BF16, kind="ExternalInput")
    qtb_d = nc.dram_tensor("qtb", [BLOC, D, L], BF16, kind="ExternalInput")
    F8 = mybir.dt.float8e4
    wr_d = nc.dram_tensor("wr", [NM, 128, KF, 128], BF16, kind="ExternalInput")
    wgx_d = nc.dram_tensor("wgx", [NM, 128, NK, 128], BF16,
                           kind="ExternalInput")
    wg8_d = nc.dram_tensor("wg8", [NM, 128, NK, 2, 128], F8,
                           kind="ExternalInput")
    brt_d = nc.dram_tensor("brt", [128, NM], FP32, kind="ExternalInput")
    bgt_d = nc.dram_tensor("bgt", [128, NM], FP32, kind="ExternalInput")
    outs_d = nc.dram_tensor("outs", [BLOC, D, L], FP32, kind="ExternalOutput")
    outq_d = nc.dram_tensor("outq", [BLOC, D, L], FP32, kind="ExternalOutput")
    ident_d = nc.inline_tensor(np.eye(128, dtype=np.float32), name="identsrc")
    identb_d = nc.inline_tensor(
        np.eye(128, dtype=ml_dtypes.bfloat16), name="identbsrc")

    WAVES = [(0, 1, 2), (3, 4, 5), (6, 7)]

    with tile.TileContext(nc) as tc:
        # consumption order of (k, ms) stationary chunks across S1 waves
        ORDER = [(k, ms) for wave in WAVES for k in range(NK) for ms in wave]
        PRE = 8   # stf ring depth = cross-batch prefetch count
        NQP = 8   # qt row tiles per batch (full ring: read in every wave)

        with (
            tc.tile_pool(name="prog", bufs=1) as Pp,
            tc.tile_pool(name="qpool", bufs=1) as Pq,
            tc.tile_pool(name="stpool", bufs=1) as Pst,
        ):
            ident = Pp.tile([128, 128], FP32, tag="ident", name="ident")
            nc.sync.dma_start(ident[:], ident_d[:])
            identb = Pp.tile([128, 128], BF16, tag="identb", name="identb")
            nc.sync.dma_start(identb[:], identb_d[:])
            brt = Pp.tile([128, NM], FP32, tag="brt", name="brt")
            nc.sync.dma_start(brt[:], brt_d[:])
            bgt = Pp.tile([128, NM], FP32, tag="bgt", name="bgt")
            nc.sync.dma_start(bgt[:], bgt_d[:])

            def qtf_dma(b, k):
                t = Pq.tile([128, L], FP32R, tag="qtf", bufs=NQP,
                            name=f"qtf{b}_{k}")
                nc.sync.dma_start(t[:], qt_d[b, k * 128:(k + 1) * 128, :])
                return t

            def stf_dma(b, k, ms):
                t = Pst.tile([128, 128], FP32R, tag="stf", bufs=PRE,
                             name=f"stf{b}_{k}_{ms}")
                nc.sync.dma_start(
                    t[:], st_d[b, k * 128:(k + 1) * 128,
                               ms * 128:(ms + 1) * 128])
                return t

            # batch 0 inputs: interleave qt rows with the first stationary
            # chunks so wave-0 matmuls chase DMA arrival
            qtf = []
            stf_pre = {}
            pi = 0
            for k in range(NK):
                qtf.append(qtf_dma(0, k))
                while pi < PRE and ORDER[pi][0] <= k + 1:
                    stf_pre[ORDER[pi]] = stf_dma(0, *ORDER[pi])
                    pi += 1

            for b in range(BLOC):
                with tc.tile_pool(name=f"batch{b}", bufs=1) as Pb:
                    stats = {}
                    for nm_ in ("negm1", "d1", "lnd1", "l1n",
                                "negm2", "d2", "lnd2", "l2n"):
                        stats[nm_] = Pb.tile([128, NK], FP32, tag=nm_,
                                             name=f"{nm_}{b}")
                    stbt = [Pb.tile([128, L], BF16, tag="stb", bufs=NK,
                                    name=f"stb{b}_{k}") for k in range(NK)]
                    qtbt = [Pb.tile([128, L], BF16, tag="qtb", bufs=NK,
                                    name=f"qtb{b}_{k}") for k in range(NK)]
                    bT = []
                    cT = []

                    with tc.tile_pool(name=f"bc{b}", bufs=1) as Pc:
                        A = [Pc.tile([128, L], FP32, tag="A", bufs=NK,
                                     name=f"A{b}_{ms}") for ms in range(NK)]
                        p1t = Pc.tile([128, NK, L], BF16, tag="p1t",
                                      name=f"p1t{b}")
                        p2 = Pc.tile([128, NK, L], BF16, tag="p2",
                                     name=f"p2{b}")

                        def chain1(ms):
                            nc.vector.tensor_reduce(
                                stats["negm1"][:, ms:ms + 1], A[ms][:], AX.X,
                                ALU.max, negate=True)
                            esc = Pc.tile([128, L], BF16, tag="esc", bufs=1,
                                          name=f"esc{b}_{ms}")
                            nc.scalar.activation(
                                esc[:], A[ms][:], AF.Exp,
                                bias=stats["negm1"][:, ms:ms + 1],
                                accum_out=stats["d1"][:, ms:ms + 1])
                            nc.scalar.activation(
                                stats["lnd1"][:, ms:ms + 1],
                                stats["d1"][:, ms:ms + 1], AF.Ln)
                            nc.vector.tensor_sub(
                                stats["l1n"][:, ms:ms + 1],
                                stats["negm1"][:, ms:ms + 1],
                                stats["lnd1"][:, ms:ms + 1])
                            p1s = Pc.tile([128, L], BF16, tag="p1s", bufs=3,
                                          name=f"p1s{b}_{ms}")
                            nc.scalar.activation(
                                p1s[:], A[ms][:], AF.Exp,
                                bias=stats["l1n"][:, ms:ms + 1])
                            return p1s

                        p1s_tiles = {}

                        def t_p1(ms, pool):
                            strip = pool.tile([128, NK, 128], BF16,
                                              tag="strip", bufs=1,
                                              name=f"strp1{b}_{ms}")
                            for mt in range(NK):
                                nc.tensor.transpose(
                                    strip[:, mt, :],
                                    p1s_tiles[ms][:, mt * 128:(mt + 1) * 128],
                                    identb[:])
                            nc.vector.tensor_copy(
                                p1t[:, 0:NK, ms * 128:(ms + 1) * 128],
                                strip[:])

                        # ---------- S1: A = S Q^T + row softmax ----------
                        with tc.tile_pool(name=f"ps1_{b}", bufs=1,
                                          space="PSUM") as PS1:
                            for wi, wave in enumerate(WAVES):
                                pa = {}
                                for ms in wave:
                                    pa[ms] = [
                                        PS1.tile([128, 512], FP32, tag="pa",
                                                 bufs=7,
                                                 name=f"pa{b}_{ms}_{h}")
                                        for h in range(NH)]
                                for k in range(NK):
                                    for ms in wave:
                                        stf = stf_pre.pop(
                                            (k, ms), None) or stf_dma(b, k, ms)
                                        for h in range(NH):
                                            nc.tensor.matmul(
                                                pa[ms][h][:], stf[:],
                                                qtf[k][:,
                                                       h * 512:(h + 1) * 512],
                                                start=(k == 0),
                                                stop=(k == NK - 1))
                                for ms in wave:
                                    for h in range(NH):
                                        nc.vector.tensor_copy(
                                            A[ms][:, h * 512:(h + 1) * 512],
                                            pa[ms][h][:])
                                if wi > 0:
                                    for ms in WAVES[wi - 1]:
                                        t_p1(ms, PS1)
                                for ms in wave:
                                    p1s_tiles[ms] = chain1(ms)

                        # ---------- B/C: A^T softmax + stage-2 ----------
                        with tc.tile_pool(name=f"psbc_{b}", bufs=1,
                                          space="PSUM") as PSb:
                            at = {}

                            def at_mk(mt):
                                t = PSb.tile([128, L], FP32, tag="at", bufs=2,
                                             name=f"at{b}_{mt}")
                                for c in range(NK):
                                    nc.tensor.transpose(
                                        t[:, c * 128:(c + 1) * 128],
                                        A[c][:, mt * 128:(mt + 1) * 128],
                                        ident[:])
                                at[mt] = t

                            def chain2(mt):
                                nc.vector.tensor_reduce(
                                    stats["negm2"][:, mt:mt + 1], at[mt][:],
                                    AX.X, ALU.max, negate=True)
                                esc = Pc.tile([128, L], BF16, tag="esc",
                                              bufs=1, name=f"esc2{b}_{mt}")
                                nc.scalar.activation(
                                    esc[:], at[mt][:], AF.Exp,
                                    bias=stats["negm2"][:, mt:mt + 1],
                                    accum_out=stats["d2"][:, mt:mt + 1])
                                nc.scalar.activation(
                                    stats["lnd2"][:, mt:mt + 1],
                                    stats["d2"][:, mt:mt + 1], AF.Ln)
                                nc.vector.tensor_sub(
                                    stats["l2n"][:, mt:mt + 1],
                                    stats["negm2"][:, mt:mt + 1],
                                    stats["lnd2"][:, mt:mt + 1])
                                p2s = Pc.tile([128, L], BF16, tag="p2s",
                                              bufs=3, name=f"p2s{b}_{mt}")
                                nc.scalar.activation(
                                    p2s[:], at[mt][:], AF.Exp,
                                    bias=stats["l2n"][:, mt:mt + 1])
                                return p2s

                            p2s_tiles = {}

                            def t_p2(mt):
                                strip = PSb.tile([128, NK, 128], BF16,
                                                 tag="strip", bufs=1,
                                                 name=f"strp2{b}_{mt}")
                                for ms in range(NK):
                                    nc.tensor.transpose(
                                        strip[:, ms, :],
                                        p2s_tiles[mt][:,
                                                      ms * 128:(ms + 1) * 128],
                                        identb[:])
                                nc.vector.tensor_copy(
                                    p2[:, 0:NK, mt * 128:(mt + 1) * 128],
                                    strip[:])

                            # warm-up: first two at tiles + S1 tail
                            # transposes (their ACT chains need the gap)
                            at_mk(0)
                            t_p1(WAVES[-1][0], PSb)
                            at_mk(1)
                            t_p1(WAVES[-1][1], PSb)
                            p2s_tiles[0] = chain2(0)
                            p2s_tiles[1] = chain2(1)

                            def bmm(md, nat_d, pmov, out_list, tagc):
                                pb = [PSb.tile([128, 512], FP32, tag="pb",
                                               bufs=3,
                                               name=f"pb{b}_{tagc}{md}_{h}")
                                      for h in range(NH)]
                                for kt in range(NK):
                                    ch = Pc.tile(
                                        [128, 128], BF16, tag=f"{tagc}chunk",
                                        bufs=16, name=f"{tagc}ch{b}_{md}_{kt}")
                                    nc.sync.dma_start(
                                        ch[:],
                                        nat_d[b, kt * 128:(kt + 1) * 128,
                                              md * 128:(md + 1) * 128])
                                    for h in range(NH):
                                        nc.tensor.matmul(
                                            pb[h][:], ch[:],
                                            pmov[:, kt, h * 512:(h + 1) * 512],
                                            start=(kt == 0),
                                            stop=(kt == NK - 1))
                                ot = Pb.tile([128, L], BF16, tag=f"{tagc}T",
                                             bufs=NM, name=f"{tagc}T{b}_{md}")
                                for h in range(NH):
                                    nc.vector.tensor_copy(
                                        ot[:, h * 512:(h + 1) * 512], pb[h][:])
                                out_list.append(ot)

                            w0 = {}
                            for md in range(NM):
                                bmm(md, qnb_d, p1t, bT, "b")
                                if md == 1:
                                    # prefetch first heuristic g weights
                                    t = Pb.tile([128, NK, 128], BF16,
                                                tag="wgx0", name=f"wgx0_{b}")
                                    nc.sync.dma_start(t[:], wgx_d[0])
                                    w0["wgx"] = t
                                    t = Pb.tile([128, NK, 2, 128], F8,
                                                tag="wg80", name=f"wg80_{b}")
                                    nc.sync.dma_start(t[:], wg8_d[0])
                                    w0["wg8"] = t
                                if md >= 1:
                                    t_p2(md - 1)
                                mt = md + 2
                                if mt < NK:
                                    at_mk(mt)
                                    p2s_tiles[mt] = chain2(mt)
                            t_p2(NK - 2)
                            t_p2(NK - 1)
                            for k in range(NK):
                                nc.sync.dma_start(
                                    stbt[k][:],
                                    stb_d[b, k * 128:(k + 1) * 128, :])
                                nc.sync.dma_start(
                                    qtbt[k][:],
                                    qtb_d[b, k * 128:(k + 1) * 128, :])
                            for md in range(NM):
                                bmm(md, snb_d, p2, cT, "c")

                    # ---------- H: heuristic ----------
                    with (
                        tc.tile_pool(name=f"heur{b}", bufs=1) as Ph,
                        tc.tile_pool(name=f"psH{b}", bufs=7,
                                     space="PSUM") as PSh,
                    ):
                        # fp8 DoubleRow pair tiles for the g branch
                        # (j 0..3 = y chunks 2j/2j+1, j 4..7 = x*y chunks);
                        # side-s y pairs first: the first g matmuls need them
                        xys = []
                        xyq = []
                        f8s = []
                        f8q = []
                        for j in range(4):
                            t = Ph.tile([128, 2, L], F8, tag="f8s", bufs=NK,
                                        name=f"f8s{b}_{j}")
                            nc.vector.tensor_copy(t[:, 0, :], bT[2 * j][:])
                            nc.vector.tensor_copy(t[:, 1, :], bT[2 * j + 1][:])
                            f8s.append(t)
                        for k in range(NK):
                            t1 = Ph.tile([128, L], BF16, tag="xys", bufs=NK,
                                         name=f"xys{b}_{k}")
                            nc.vector.tensor_mul(t1[:], stbt[k][:], bT[k][:])
                            xys.append(t1)
                        for j in range(4):
                            t = Ph.tile([128, 2, L], F8, tag="f8s", bufs=NK,
                                        name=f"f8s{b}_{4 + j}")
                            nc.vector.tensor_copy(t[:, 0, :], xys[2 * j][:])
                            nc.vector.tensor_copy(t[:, 1, :],
                                                  xys[2 * j + 1][:])
                            f8s.append(t)
                        for j in range(4):
                            t = Ph.tile([128, 2, L], F8, tag="f8q", bufs=NK,
                                        name=f"f8q{b}_{j}")
                            nc.vector.tensor_copy(t[:, 0, :], cT[2 * j][:])
                            nc.vector.tensor_copy(t[:, 1, :], cT[2 * j + 1][:])
                            f8q.append(t)
                        for k in range(NK):
                            t2 = Ph.tile([128, L], BF16, tag="xyq", bufs=NK,
                                         name=f"xyq{b}_{k}")
                            nc.vector.tensor_mul(t2[:], qtbt[k][:], cT[k][:])
                            xyq.append(t2)
                        for j in range(4):
                            t = Ph.tile([128, 2, L], F8, tag="f8q", bufs=NK,
                                        name=f"f8q{b}_{4 + j}")
                            nc.vector.tensor_copy(t[:, 0, :], xyq[2 * j][:])
                            nc.vector.tensor_copy(t[:, 1, :],
                                                  xyq[2 * j + 1][:])
                            f8q.append(t)

                        for m in range(NM):
                            wrt = Ph.tile([128, KF, 128], BF16, tag="wr",
                                          bufs=2, name=f"wrt{b}_{m}")
                            nc.sync.dma_start(wrt[:], wr_d[m])
                            if m == 0:
                                wgxt, wg8t = w0["wgx"], w0["wg8"]
                            else:
                                wgxt = Ph.tile([128, NK, 128], BF16,
                                               tag="wgx", bufs=2,
                                               name=f"wgx{b}_{m}")
                                nc.sync.dma_start(wgxt[:], wgx_d[m])
                                wg8t = Ph.tile([128, NK, 2, 128], F8,
                                               tag="wg8", bufs=2,
                                               name=f"wg8{b}_{m}")
                                nc.sync.dma_start(wg8t[:], wg8_d[m])
                            if m == 0 and b + 1 < BLOC:
                                qtf = [qtf_dma(b + 1, k) for k in range(NQP)]
                                for i in range(PRE):
                                    stf_pre[ORDER[i]] = stf_dma(
                                        b + 1, *ORDER[i])
                            for xt, blocks, f8p, outd in (
                                (stbt, (stbt, bT, xys), f8s, outs_d),
                                (qtbt, (qtbt, cT, xyq), f8q, outq_d),
                            ):
                                tag = "s" if outd is outs_d else "q"
                                pg = [PSh.tile([128, 512], FP32, tag="rg",
                                               bufs=7,
                                               name=f"pg{b}_{m}{tag}{h}")
                                      for h in range(NH)]
                                pr = [PSh.tile([128, 512], FP32, tag="rg",
                                               bufs=7,
                                               name=f"pr{b}_{m}{tag}{h}")
                                      for h in range(NH)]
                                # g branch: x block bf16, y/x*y fp8 DoubleRow
                                for kf in range(NK):
                                    rhs = blocks[0][kf]
                                    for h in range(NH):
                                        nc.tensor.matmul(
                                            pg[h][:], wgxt[:, kf, :],
                                            rhs[:, h * 512:(h + 1) * 512],
                                            start=(kf == 0), stop=False)
                                for j in range(NK):
                                    for h in range(NH):
                                        nc.tensor.matmul(
                                            pg[h][:], wg8t[:, j, :, :],
                                            f8p[j][:, :,
                                                   h * 512:(h + 1) * 512],
                                            start=False, stop=(j == NK - 1),
                                            perf_mode=(
                                                mybir.MatmulPerfMode
                                                .DoubleRow))
                                # r branch: all bf16; sigmoid overlaps
                                g_sb = Ph.tile([128, L], BF16, tag="gsb",
                                               bufs=1, name=f"gsb{b}_{m}{tag}")
                                for h in range(NH):
                                    nc.scalar.activation(
                                        g_sb[:, h * 512:(h + 1) * 512],
                                        pg[h][:], AF.Sigmoid,
                                        bias=bgt[:, m:m + 1])
                                for kf in range(KF):
                                    rhs = blocks[kf // NK][kf % NK]
                                    for h in range(NH):
                                        nc.tensor.matmul(
                                            pr[h][:], wrt[:, kf, :],
                                            rhs[:, h * 512:(h + 1) * 512],
                                            start=(kf == 0),
                                            stop=(kf == KF - 1))
                                r_sb = Ph.tile([128, L], BF16, tag="rsb",
                                               bufs=1, name=f"rsb{b}_{m}{tag}")
                                for h in range(NH):
                                    nc.scalar.activation(
                                        r_sb[:, h * 512:(h + 1) * 512],
                                        pr[h][:], AF.Gelu_apprx_tanh,
                                        bias=brt[:, m:m + 1])
                                t1 = Ph.tile([128, L], FP32, tag="t1", bufs=1,
                                             name=f"t1{b}_{m}{tag}")
                                nc.vector.tensor_sub(t1[:], r_sb[:], xt[m][:])
                                t2 = Ph.tile([128, L], BF16, tag="t2", bufs=1,
                                             name=f"t2{b}_{m}{tag}")
                                nc.vector.tensor_mul(t2[:], g_sb[:], t1[:])
                                osb = Ph.tile([128, L], FP32, tag="osb",
                                              bufs=1, name=f"osb{b}_{m}{tag}")
                                nc.vector.tensor_add(osb[:], t2[:], xt[m][:])
                                nc.sync.dma_start(
                                    outd[b, m * 128:(m + 1) * 128, :], osb[:])

    nc.compile()
    return nc


def _get_nc():
    global _nc_cache
    if _nc_cache is None:
        _nc_cache = _build()
    return _nc_cache


def _prep_inputs(s, q, w_r, b_r, w_g, b_g):
    bf = ml_dtypes.bfloat16
    s = np.ascontiguousarray(np.asarray(s, dtype=np.float32))
    q = np.ascontiguousarray(np.asarray(q, dtype=np.float32))
    w_r = np.asarray(w_r, dtype=np.float32)
    w_g = np.asarray(w_g, dtype=np.float32)
    b_r = np.asarray(b_r, dtype=np.float32)
    b_g = np.asarray(b_g, dtype=np.float32)

    st = np.ascontiguousarray(s.transpose(0, 2, 1))
    qt = np.ascontiguousarray(q.transpose(0, 2, 1))
    snb = s.astype(bf)
    qnb = q.astype(bf)
    stb = st.astype(bf)
    qtb = qt.astype(bf)

    def fold_w(w):
        W1, W2, W3, W4 = (w[:, i * D:(i + 1) * D] for i in range(4))
        eff = np.concatenate([W1 + W4, W2 - W4, W3], axis=1)  # [D, 3D]
        return eff.T  # [3D, D] contraction-major

    wt_r = fold_w(w_r)
    wr_pack = np.ascontiguousarray(
        wt_r.reshape(KF, 128, NM, 128).transpose(2, 1, 0, 3)).astype(bf)

    wt_g = fold_w(w_g)
    # g: x block bf16 [m, f, k, o]; y/x*y blocks fp8 DoubleRow pairs
    # [m, f, j, i, o] with contraction chunk = 8 + 2j + i
    wgx_pack = np.ascontiguousarray(
        wt_g[0:D].reshape(NK, 128, NM, 128).transpose(2, 1, 0, 3)).astype(bf)
    f8 = ml_dtypes.float8_e4m3
    wg8_pack = np.ascontiguousarray(
        wt_g[D:3 * D].reshape(NK, 2, 128, NM, 128)
        .transpose(3, 2, 0, 1, 4)).astype(f8)
    brt = np.ascontiguousarray(b_r.reshape(NM, 128).T)
    bgt = np.ascontiguousarray(b_g.reshape(NM, 128).T)

    in_maps = []
    for c in range(NCORES):
        sl = slice(BLOC * c, BLOC * (c + 1))
        in_maps.append({
            "st": st[sl], "qt": qt[sl],
            "snb": snb[sl], "qnb": qnb[sl],
            "stb": stb[sl], "qtb": qtb[sl],
            "wr": wr_pack, "wgx": wgx_pack, "wg8": wg8_pack,
            "brt": brt, "bgt": bgt,
        })
    return in_maps


def run(inputs, trace=False, tmpdir=None):
    """Execute on 8 NeuronCores; returns ((s_tilde, q_tilde), BassKernelResults)."""
    from concourse.bass_utils import run_bass_kernel_spmd

    in_maps = _prep_inputs(
        inputs["s"], inputs["q"], inputs["w_r"], inputs["b_r"],
        inputs["w_g"], inputs["b_g"])
    nc = _get_nc()
    res = run_bass_kernel_spmd(nc, in_maps, list(range(NCORES)), trace=trace,
                               tmpdir=tmpdir)
    s_t = np.empty((B, L, D), np.float32)
    q_t = np.empty((B, L, D), np.float32)
    for c in range(NCORES):
        sl = slice(BLOC * c, BLOC * (c + 1))
        s_t[sl] = res.results[c]["outs"].transpose(0, 2, 1)
        q_t[sl] = res.results[c]["outq"].transpose(0, 2, 1)
    return (s_t, q_t), res


def kernel(s, q, w_r, b_r, w_g, b_g, s_mask=None, q_mask=None):
    # s_mask / q_mask are all-ones in this problem; the additive mask term
    # (1 - m1*m2) * NEG_INF is identically zero, so they are unused.
    out, _ = run({"s": s, "q": q, "w_r": w_r, "b_r": b_r,
                  "w_g": w_g, "b_g": b_g})
    return out


# revision 32
# speedup vs baseline: 1.3631x; 1.0367x over previous
"""Trainium2 Bass kernel for nn_Attention_65223373357517.

Computes, for s,q [B=16, L=1024, D=1024] (D = 2H, H=512):
    a  = einsum('bsd,btd->bst', s, q)
    b  = softmax(a, -1) @ q
    c  = softmax(a^T, -1) @ s
    s~ = heuristic(s, b);  q~ = heuristic(q, c)
with heuristic(x, y) = g*r + (1-g)*x,
    r = gelu_tanh([x, y, x*y, x-y] @ w_r.T + b_r)
    g = sigmoid ([x, y, x*y, x-y] @ w_g.T + b_g)

Strategy: pure data-parallel over batch (2 examples per NeuronCore, 8 cores,
no collectives). Host folds the (x-y) block into the x/y weight blocks
(W1+W4, W2-W4, W3), transposes activations so every on-chip matmul is in
its natural layout, and transposes outputs back.  Masks are all-ones in
this problem configuration (additive mask term is identically zero), so
they do not enter the computation.

On-chip per batch (v2 schedule — PE kept saturated):
  S1:  A = S Q^T via f32r matmuls.  qt rows resident (prefetched during
       the previous batch's heuristic); st streamed as [128,128]
       stationary chunks, each reused for both 512-halves.  ms processed
       in waves of 3 (PSUM bound), k inner so compute chases DMA.
       Per A tile: row max m1, d1 = sum exp(A-m1) (fused ACT accum),
       l1 = m1 + ln d1 folded into the ACT bias so P1 = exp(A - l1) is
       the *normalized* softmax in A layout (bf16, per-partition bias —
       no cross-partition broadcast needed).  PE bf16-transposes P1 into
       p1t strips, issued one wave late so the ACT chain stays hidden.
  B/C: A^T tiles via PE fp32 transposes (PSUM); same per-tile stat chain
       gives P2^T = exp(A^T - l2) bf16, transposed back into p2 strips.
       b^T = Q_nat-chunk-contracted matmuls with rhs p1t; c^T likewise
       with lhsT = S_nat chunks, rhs p2.  at/p2 transposes interleave
       with the b^T matmul stream so PE never waits on stat chains.
  H:   heuristic per 128-row output strip: 24 K-chunk bf16 matmuls each
       for r and g over blocks [x^T, y^T, (x*y)^T]; gelu/sigmoid read
       PSUM with per-partition bias; epilogue out = x + g*(r - x);
       stream out s~^T / q~^T.  Next batch's qt prefetches here.
"""

import numpy as np
import ml_dtypes

B, L, D = 16, 1024, 1024
NCORES = 8
BLOC = B // NCORES          # batches per core
NK = D // 128               # contraction chunks (8)
NM = D // 128               # output-row chunks (8)
KF = 3 * D // 128           # folded heuristic contraction chunks (24)
NH = 2                      # 512-wide halves of a 1024 free dim

_nc_cache = None


def _build():
    import concourse.tile as tile
    from concourse import bacc, mybir

    FP32 = mybir.dt.float32
    FP32R = mybir.dt.float32r
    BF16 = mybir.dt.bfloat16
    AF = mybir.ActivationFunctionType
    ALU = mybir.AluOpType
    AX = mybir.AxisListType

    nc = bacc.Bacc("TRN2", target_bir_lowering=False, debug=False)

    st_d = nc.dram_tensor("st", [BLOC, D, L], FP32R, kind="ExternalInput")
    qt_d = nc.dram_tensor("qt", [BLOC, D, L], FP32R, kind="ExternalInput")
    snb_d = nc.dram_tensor("snb", [BLOC, L, D], BF16, kind="ExternalInput")
    qnb_d = nc.dram_tensor("qnb", [BLOC, L, D], BF16, kind="ExternalInput")
    stb_d = nc.dram_tensor("stb", [BLOC, D, L], BF16, kind="ExternalInput")
    qtb_d = nc.dram_tensor("qtb", [BLOC, D, L], BF16, kind="ExternalInput")
    F8 = mybir.dt.float8e4
    wr_d = nc.dram_tensor("wr", [NM, 128, KF, 128], BF16, kind="ExternalInput")
    wgx_d = nc.dram_tensor("wgx", [NM, 128, NK, 128], BF16,
                           kind="ExternalInput")
    wg8_d = nc.dram_tensor("wg8", [NM, 128, NK, 2, 128], F8,
                           kind="ExternalInput")
    brt_d = nc.dram_tensor("brt", [128, NM], FP32, kind="ExternalInput")
    bgt_d = nc.dram_tensor("bgt", [128, NM], FP32, kind="ExternalInput")
    outs_d = nc.dram_tensor("outs", [BLOC, D, L], FP32, kind="ExternalOutput")
    outq_d = nc.dram_tensor("outq", [BLOC, D, L], FP32, kind="ExternalOutput")
    ident_d = nc.inline_tensor(np.eye(128, dtype=np.float32), name="identsrc")
    identb_d = nc.inline_tensor(
        np.eye(128, dtype=ml_dtypes.bfloat16), name="identbsrc")

    WAVES = [(0, 1, 2), (3, 4, 5), (6, 7)]

    with tile.TileContext(nc) as tc:
        # consumption order of (k, ms) stationary chunks across S1 waves
        ORDER = [(k, ms) for wave in WAVES for k in range(NK) for ms in wave]
        PRE = 8   # stf ring depth = cross-batch prefetch count
        NQP = 8   # qt row tiles per batch (full ring: read in every wave)

        with (
            tc.tile_pool(name="prog", bufs=1) as Pp,
            tc.tile_pool(name="qpool", bufs=1) as Pq,
            tc.tile_pool(name="stpool", bufs=1) as Pst,
        ):
            ident = Pp.tile([128, 128], FP32, tag="ident", name="ident")
            nc.sync.dma_start(ident[:], ident_d[:])
            identb = Pp.tile([128, 128], BF16, tag="identb", name="identb")
            nc.sync.dma_start(identb[:], identb_d[:])
            brt = Pp.tile([128, NM], FP32, tag="brt", name="brt")
            nc.sync.dma_start(brt[:], brt_d[:])
            bgt = Pp.tile([128, NM], FP32, tag="bgt", name="bgt")
            nc.sync.dma_start(bgt[:], bgt_d[:])

            def qtf_dma(b, k):
                t = Pq.tile([128, L], FP32R, tag="qtf", bufs=NQP,
                            name=f"qtf{b}_{k}")
                nc.sync.dma_start(t[:], qt_d[b, k * 128:(k + 1) * 128, :])
                return t

            def stf_dma(b, k, ms):
                t = Pst.tile([128, 128], FP32R, tag="stf", bufs=PRE,
                             name=f"stf{b}_{k}_{ms}")
                nc.sync.dma_start(
                    t[:], st_d[b, k * 128:(k + 1) * 128,
                               ms * 128:(ms + 1) * 128])
                return t

            # batch 0 inputs: interleave qt rows with the first stationary
            # chunks so wave-0 matmuls chase DMA arrival
            qtf = []
            stf_pre = {}
            pi = 0
            for k in range(NK):
                qtf.append(qtf_dma(0, k))
                while pi < PRE and ORDER[pi][0] <= k + 1:
                    stf_pre[ORDER[pi]] = stf_dma(0, *ORDER[pi])
                    pi += 1

            for b in range(BLOC):
                with tc.tile_pool(name=f"batch{b}", bufs=1) as Pb:
                    stats = {}
                    for nm_ in ("negm1", "d1", "rd1",
                                "negm2", "d2", "rd2"):
                        stats[nm_] = Pb.tile([128, NK], FP32, tag=nm_,
                                             name=f"{nm_}{b}")
                    stbt = [Pb.tile([128, L], BF16, tag="stb", bufs=NK,
                                    name=f"stb{b}_{k}") for k in range(NK)]
                    qtbt = [Pb.tile([128, L], BF16, tag="qtb", bufs=NK,
                                    name=f"qtb{b}_{k}") for k in range(NK)]
                    bT = []
                    cT = []

                    with tc.tile_pool(name=f"bc{b}", bufs=1) as Pc:
                        A = [Pc.tile([128, L], FP32, tag="A", bufs=NK,
                                     name=f"A{b}_{ms}") for ms in range(NK)]
                        p1t = Pc.tile([128, NK, L], BF16, tag="p1t",
                                      name=f"p1t{b}")
                        p2 = Pc.tile([128, NK, L], BF16, tag="p2",
                                     name=f"p2{b}")

                        def chain1a(ms):
                            # row max + E1 = exp(A - m1) with d1 row-sums
                            nc.vector.tensor_reduce(
                                stats["negm1"][:, ms:ms + 1], A[ms][:], AX.X,
                                ALU.max, negate=True)
                            e1 = Pc.tile([128, L], BF16, tag="e1", bufs=4,
                                         name=f"e1_{b}_{ms}")
                            nc.scalar.activation(
                                e1[:], A[ms][:], AF.Exp,
                                bias=stats["negm1"][:, ms:ms + 1],
                                accum_out=stats["d1"][:, ms:ms + 1])
                            return e1

                        def chain1b(ms):
                            # P1 = E1 / d1 (normalized softmax, bf16)
                            p1s = Pc.tile([128, L], BF16, tag="p1s", bufs=3,
                                          name=f"p1s{b}_{ms}")
                            nc.vector.tensor_scalar_mul(
                                p1s[:], e1_tiles[ms][:],
                                stats["rd1"][:, ms:ms + 1])
                            return p1s

                        e1_tiles = {}
                        p1s_tiles = {}

                        def t_p1(ms, pool):
                            strip = pool.tile([128, NK, 128], BF16,
                                              tag="strip", bufs=1,
                                              name=f"strp1{b}_{ms}")
                            for mt in range(NK):
                                nc.tensor.transpose(
                                    strip[:, mt, :],
                                    p1s_tiles[ms][:, mt * 128:(mt + 1) * 128],
                                    identb[:])
                            nc.vector.tensor_copy(
                                p1t[:, 0:NK, ms * 128:(ms + 1) * 128],
                                strip[:])

                        # ---------- S1: A = S Q^T + row softmax ----------
                        with tc.tile_pool(name=f"ps1_{b}", bufs=1,
                                          space="PSUM") as PS1:
                            for wi, wave in enumerate(WAVES):
                                pa = {}
                                for ms in wave:
                                    pa[ms] = [
                                        PS1.tile([128, 512], FP32, tag="pa",
                                                 bufs=7,
                                                 name=f"pa{b}_{ms}_{h}")
                                        for h in range(NH)]
                                for k in range(NK):
                                    for ms in wave:
                                        stf = stf_pre.pop(
                                            (k, ms), None) or stf_dma(b, k, ms)
                                        for h in range(NH):
                                            nc.tensor.matmul(
                                                pa[ms][h][:], stf[:],
                                                qtf[k][:,
                                                       h * 512:(h + 1) * 512],
                                                start=(k == 0),
                                                stop=(k == NK - 1))
                                for ms in wave:
                                    for h in range(NH):
                                        nc.vector.tensor_copy(
                                            A[ms][:, h * 512:(h + 1) * 512],
                                            pa[ms][h][:])
                                if wi > 0:
                                    pw = WAVES[wi - 1]
                                    nc.vector.reciprocal(
                                        stats["rd1"][:, pw[0]:pw[-1] + 1],
                                        stats["d1"][:, pw[0]:pw[-1] + 1])
                                    for ms in pw:
                                        p1s_tiles[ms] = chain1b(ms)
                                        t_p1(ms, PS1)
                                for ms in wave:
                                    e1_tiles[ms] = chain1a(ms)

                        # ---------- B/C: A^T softmax + stage-2 ----------
                        with tc.tile_pool(name=f"psbc_{b}", bufs=1,
                                          space="PSUM") as PSb:
                            at = {}

                            def at_mk(mt):
                                t = PSb.tile([128, L], FP32, tag="at", bufs=2,
                                             name=f"at{b}_{mt}")
                                for c in range(NK):
                                    nc.tensor.transpose(
                                        t[:, c * 128:(c + 1) * 128],
                                        A[c][:, mt * 128:(mt + 1) * 128],
                                        ident[:])
                                at[mt] = t

                            def chain2a(mt):
                                nc.vector.tensor_reduce(
                                    stats["negm2"][:, mt:mt + 1], at[mt][:],
                                    AX.X, ALU.max, negate=True)
                                e2 = Pc.tile([128, L], BF16, tag="e1",
                                             bufs=4, name=f"e2_{b}_{mt}")
                                nc.scalar.activation(
                                    e2[:], at[mt][:], AF.Exp,
                                    bias=stats["negm2"][:, mt:mt + 1],
                                    accum_out=stats["d2"][:, mt:mt + 1])
                                return e2

                            def chain2b(mt):
                                p2s = Pc.tile([128, L], BF16, tag="p2s",
                                              bufs=3, name=f"p2s{b}_{mt}")
                                nc.vector.tensor_scalar_mul(
                                    p2s[:], e2_tiles[mt][:],
                                    stats["rd2"][:, mt:mt + 1])
                                return p2s

                            e2_tiles = {}
                            p2s_tiles = {}

                            def t_p2(mt):
                                strip = PSb.tile([128, NK, 128], BF16,
                                                 tag="strip", bufs=1,
                                                 name=f"strp2{b}_{mt}")
                                for ms in range(NK):
                                    nc.tensor.transpose(
                                        strip[:, ms, :],
                                        p2s_tiles[mt][:,
                                                      ms * 128:(ms + 1) * 128],
                                        identb[:])
                                nc.vector.tensor_copy(
                                    p2[:, 0:NK, mt * 128:(mt + 1) * 128],
                                    strip[:])

                            # warm-up: first two at tiles + S1 tail
                            # transposes (their ACT chains need the gap)
                            lw = WAVES[-1]
                            at_mk(0)
                            nc.vector.reciprocal(
                                stats["rd1"][:, lw[0]:lw[-1] + 1],
                                stats["d1"][:, lw[0]:lw[-1] + 1])
                            p1s_tiles[lw[0]] = chain1b(lw[0])
                            t_p1(lw[0], PSb)
                            at_mk(1)
                            p1s_tiles[lw[1]] = chain1b(lw[1])
                            t_p1(lw[1], PSb)
                            e2_tiles[0] = chain2a(0)
                            e2_tiles[1] = chain2a(1)

                            def bmm(md, nat_d, pmov, out_list, tagc):
                                pb = [PSb.tile([128, 512], FP32, tag="pb",
                                               bufs=3,
                                               name=f"pb{b}_{tagc}{md}_{h}")
                                      for h in range(NH)]
                                for kt in range(NK):
                                    ch = Pc.tile(
                                        [128, 128], BF16, tag=f"{tagc}chunk",
                                        bufs=16, name=f"{tagc}ch{b}_{md}_{kt}")
                                    nc.sync.dma_start(
                                        ch[:],
                                        nat_d[b, kt * 128:(kt + 1) * 128,
                                              md * 128:(md + 1) * 128])
                                    for h in range(NH):
                                        nc.tensor.matmul(
                                            pb[h][:], ch[:],
                                            pmov[:, kt, h * 512:(h + 1) * 512],
                                            start=(kt == 0),
                                            stop=(kt == NK - 1))
                                ot = Pb.tile([128, L], BF16, tag=f"{tagc}T",
                                             bufs=NM, name=f"{tagc}T{b}_{md}")
                                for h in range(NH):
                                    nc.vector.tensor_copy(
                                        ot[:, h * 512:(h + 1) * 512], pb[h][:])
                                out_list.append(ot)

                            w0 = {}
                            for md in range(NM):
                                bmm(md, qnb_d, p1t, bT, "b")
                                if md == 1:
                                    # prefetch first heuristic g weights
                                    t = Pb.tile([128, NK, 128], BF16,
                                                tag="wgx0", name=f"wgx0_{b}")
                                    nc.sync.dma_start(t[:], wgx_d[0])
                                    w0["wgx"] = t
                                    t = Pb.tile([128, NK, 2, 128], F8,
                                                tag="wg80", name=f"wg80_{b}")
                                    nc.sync.dma_start(t[:], wg8_d[0])
                                    w0["wg8"] = t
                                if md >= 1:
                                    mt_t = md - 1
                                    if mt_t % 2 == 0:
                                        nc.vector.reciprocal(
                                            stats["rd2"][:, mt_t:mt_t + 2],
                                            stats["d2"][:, mt_t:mt_t + 2])
                                    p2s_tiles[mt_t] = chain2b(mt_t)
                                    t_p2(mt_t)
                                mt = md + 2
                                if mt < NK:
                                    at_mk(mt)
                                    e2_tiles[mt] = chain2a(mt)
                            p2s_tiles[NK - 1] = chain2b(NK - 1)
                            t_p2(NK - 1)
                            for k in range(NK):
                                nc.sync.dma_start(
                                    stbt[k][:],
                                    stb_d[b, k * 128:(k + 1) * 128, :])
                                nc.sync.dma_start(
                                    qtbt[k][:],
                                    qtb_d[b, k * 128:(k + 1) * 128, :])
                            for md in range(NM):
                                bmm(md, snb_d, p2, cT, "c")

                    # ---------- H: heuristic ----------
                    with (
                        tc.tile_pool(name=f"heur{b}", bufs=1) as Ph,
                        tc.tile_pool(name=f"psH{b}", bufs=7,
                                     space="PSUM") as PSh,
                    ):
                        # fp8 DoubleRow pair tiles for the g branch
                        # (j 0..3 = y chunks 2j/2j+1, j 4..7 = x*y chunks);
                        # side-s y pairs first: the first g matmuls need them
                        xys = []
                        xyq = []
                        f8s = []
                        f8q = []
                        for j in range(4):
                            t = Ph.tile([128, 2, L], F8, tag="f8s", bufs=NK,
                                        name=f"f8s{b}_{j}")
                            nc.vector.tensor_copy(t[:, 0, :], bT[2 * j][:])
                            nc.vector.tensor_copy(t[:, 1, :], bT[2 * j + 1][:])
                            f8s.append(t)
                        for k in range(NK):
                            t1 = Ph.tile([128, L], BF16, tag="xys", bufs=NK,
                                         name=f"xys{b}_{k}")
                            nc.vector.tensor_mul(t1[:], stbt[k][:], bT[k][:])
                            xys.append(t1)
                        for j in range(4):
                            t = Ph.tile([128, 2, L], F8, tag="f8s", bufs=NK,
                                        name=f"f8s{b}_{4 + j}")
                            nc.vector.tensor_copy(t[:, 0, :], xys[2 * j][:])
                            nc.vector.tensor_copy(t[:, 1, :],
                                                  xys[2 * j + 1][:])
                            f8s.append(t)
                        for j in range(4):
                            t = Ph.tile([128, 2, L], F8, tag="f8q", bufs=NK,
                                        name=f"f8q{b}_{j}")
                            nc.vector.tensor_copy(t[:, 0, :], cT[2 * j][:])
                            nc.vector.tensor_copy(t[:, 1, :], cT[2 * j + 1][:])
                            f8q.append(t)
                        for k in range(NK):
                            t2 = Ph.tile([128, L], BF16, tag="xyq", bufs=NK,
                                         name=f"xyq{b}_{k}")
                            nc.vector.tensor_mul(t2[:], qtbt[k][:], cT[k][:])
                            xyq.append(t2)
                        for j in range(4):
                            t = Ph.tile([128, 2, L], F8, tag="f8q", bufs=NK,
                                        name=f"f8q{b}_{4 + j}")
                            nc.vector.tensor_copy(t[:, 0, :], xyq[2 * j][:])
                            nc.vector.tensor_copy(t[:, 1, :],
                                                  xyq[2 * j + 1][:])
                            f8q.append(t)

                        for m in range(NM):
                            wrt = Ph.tile([128, KF, 128], BF16, tag="wr",
                                          bufs=2, name=f"wrt{b}_{m}")
                            nc.sync.dma_start(wrt[:], wr_d[m])
                            if m == 0:
                                wgxt, wg8t = w0["wgx"], w0["wg8"]
                            else:
                                wgxt = Ph.tile([128, NK, 128], BF16,
                                               tag="wgx", bufs=2,
                                               name=f"wgx{b}_{m}")
                                nc.sync.dma_start(wgxt[:], wgx_d[m])
                                wg8t = Ph.tile([128, NK, 2, 128], F8,
                                               tag="wg8", bufs=2,
                                               name=f"wg8{b}_{m}")
                                nc.sync.dma_start(wg8t[:], wg8_d[m])
                            if m == 0 and b + 1 < BLOC:
                                qtf = [qtf_dma(b + 1, k) for k in range(NQP)]
                                for i in range(PRE):
                                    stf_pre[ORDER[i]] = stf_dma(
                                        b + 1, *ORDER[i])
                            for xt, blocks, f8p, outd in (
                                (stbt, (stbt, bT, xys), f8s, outs_d),
                                (qtbt, (qtbt, cT, xyq), f8q, outq_d),
                            ):
                                tag = "s" if outd is outs_d else "q"
                                pg = [PSh.tile([128, 512], FP32, tag="rg",
                                               bufs=7,
                                               name=f"pg{b}_{m}{tag}{h}")
                                      for h in range(NH)]
                                pr = [PSh.tile([128, 512], FP32, tag="rg",
                                               bufs=7,
                                               name=f"pr{b}_{m}{tag}{h}")
                                      for h in range(NH)]
                                # g branch: x block bf16, y/x*y fp8 DoubleRow
                                for kf in range(NK):
                                    rhs = blocks[0][kf]
                                    for h in range(NH):
                                        nc.tensor.matmul(
                                            pg[h][:], wgxt[:, kf, :],
                                            rhs[:, h * 512:(h + 1) * 512],
                                            start=(kf == 0), stop=False)
                                for j in range(NK):
                                    for h in range(NH):
                                        nc.tensor.matmul(
                                            pg[h][:], wg8t[:, j, :, :],
                                            f8p[j][:, :,
                                                   h * 512:(h + 1) * 512],
                                            start=False, stop=(j == NK - 1),
                                            perf_mode=(
                                                mybir.MatmulPerfMode
                                                .DoubleRow))
                                # r branch: all bf16; sigmoid overlaps
                                g_sb = Ph.tile([128, L], BF16, tag="gsb",
                                               bufs=1, name=f"gsb{b}_{m}{tag}")
                                for h in range(NH):
                                    nc.scalar.activation(
                                        g_sb[:, h * 512:(h + 1) * 512],
                                        pg[h][:], AF.Sigmoid,
                                        bias=bgt[:, m:m + 1])
                                r_sb = Ph.tile([128, L], BF16, tag="rsb",
                                               bufs=1, name=f"rsb{b}_{m}{tag}")
                                t1 = Ph.tile([128, L], FP32, tag="t1", bufs=1,
                                             name=f"t1{b}_{m}{tag}")
                                t2 = Ph.tile([128, L], BF16, tag="t2", bufs=1,
                                             name=f"t2{b}_{m}{tag}")
                                osb = Ph.tile([128, L], FP32, tag="osb",
                                              bufs=1, name=f"osb{b}_{m}{tag}")
                                split = (m == NM - 1)
                                hgroups = ([(0,), (1,)] if split
                                           else [(0, 1)])
                                for hg in hgroups:
                                    for kf in range(KF):
                                        rhs = blocks[kf // NK][kf % NK]
                                        for h in hg:
                                            nc.tensor.matmul(
                                                pr[h][:], wrt[:, kf, :],
                                                rhs[:, h * 512:(h + 1) * 512],
                                                start=(kf == 0),
                                                stop=(kf == KF - 1))
                                    for h in hg:
                                        sl = slice(h * 512, (h + 1) * 512)
                                        nc.scalar.activation(
                                            r_sb[:, sl], pr[h][:],
                                            AF.Gelu_apprx_tanh,
                                            bias=brt[:, m:m + 1])
                                        nc.vector.tensor_sub(
                                            t1[:, sl], r_sb[:, sl],
                                            xt[m][:, sl])
                                        nc.vector.tensor_mul(
                                            t2[:, sl], g_sb[:, sl], t1[:, sl])
                                        nc.vector.tensor_add(
                                            osb[:, sl], t2[:, sl],
                                            xt[m][:, sl])
                                        nc.sync.dma_start(
                                            outd[b, m * 128:(m + 1) * 128,
                                                 sl], osb[:, sl])

    nc.compile()
    return nc


def _get_nc():
    global _nc_cache
    if _nc_cache is None:
        _nc_cache = _build()
    return _nc_cache


def _prep_inputs(s, q, w_r, b_r, w_g, b_g):
    bf = ml_dtypes.bfloat16
    s = np.ascontiguousarray(np.asarray(s, dtype=np.float32))
    q = np.ascontiguousarray(np.asarray(q, dtype=np.float32))
    w_r = np.asarray(w_r, dtype=np.float32)
    w_g = np.asarray(w_g, dtype=np.float32)
    b_r = np.asarray(b_r, dtype=np.float32)
    b_g = np.asarray(b_g, dtype=np.float32)

    st = np.ascontiguousarray(s.transpose(0, 2, 1))
    qt = np.ascontiguousarray(q.transpose(0, 2, 1))
    snb = s.astype(bf)
    qnb = q.astype(bf)
    stb = st.astype(bf)
    qtb = qt.astype(bf)

    def fold_w(w):
        W1, W2, W3, W4 = (w[:, i * D:(i + 1) * D] for i in range(4))
        eff = np.concatenate([W1 + W4, W2 - W4, W3], axis=1)  # [D, 3D]
        return eff.T  # [3D, D] contraction-major

    wt_r = fold_w(w_r)
    wr_pack = np.ascontiguousarray(
        wt_r.reshape(KF, 128, NM, 128).transpose(2, 1, 0, 3)).astype(bf)

    wt_g = fold_w(w_g)
    # g: x block bf16 [m, f, k, o]; y/x*y blocks fp8 DoubleRow pairs
    # [m, f, j, i, o] with contraction chunk = 8 + 2j + i
    wgx_pack = np.ascontiguousarray(
        wt_g[0:D].reshape(NK, 128, NM, 128).transpose(2, 1, 0, 3)).astype(bf)
    f8 = ml_dtypes.float8_e4m3
    wg8_pack = np.ascontiguousarray(
        wt_g[D:3 * D].reshape(NK, 2, 128, NM, 128)
        .transpose(3, 2, 0, 1, 4)).astype(f8)
    brt = np.ascontiguousarray(b_r.reshape(NM, 128).T)
    bgt = np.ascontiguousarray(b_g.reshape(NM, 128).T)

    in_maps = []
    for c in range(NCORES):
        sl = slice(BLOC * c, BLOC * (c + 1))
        in_maps.append({
            "st": st[sl], "qt": qt[sl],
            "snb": snb[sl], "qnb": qnb[sl],
            "stb": stb[sl], "qtb": qtb[sl],
            "wr": wr_pack, "wgx": wgx_pack, "wg8": wg8_pack,
            "brt": brt, "bgt": bgt,
        })
    return in_maps


def run(inputs, trace=False, tmpdir=None):
    """Execute on 8 NeuronCores; returns ((s_tilde, q_tilde), BassKernelResults)."""
    from concourse.bass_utils import run_bass_kernel_spmd

    in_maps = _prep_inputs(
        inputs["s"], inputs["q"], inputs["w_r"], inputs["b_r"],
        inputs["w_g"], inputs["b_g"])
    nc = _get_nc()
    res = run_bass_kernel_spmd(nc, in_maps, list(range(NCORES)), trace=trace,
                               tmpdir=tmpdir)
    s_t = np.empty((B, L, D), np.float32)
    q_t = np.empty((B, L, D), np.float32)
    for c in range(NCORES):
        sl = slice(BLOC * c, BLOC * (c + 1))
        s_t[sl] = res.results[c]["outs"].transpose(0, 2, 1)
        q_t[sl] = res.results[c]["outq"].transpose(0, 2, 1)
    return (s_t, q_t), res


def kernel(s, q, w_r, b_r, w_g, b_g, s_mask=None, q_mask=None):
    # s_mask / q_mask are all-ones in this problem; the additive mask term
    # (1 - m1*m2) * NEG_INF is identically zero, so they are unused.
    out, _ = run({"s": s, "q": q, "w_r": w_r, "b_r": b_r,
                  "w_g": w_g, "b_g": b_g})
    return out


# revision 33
# speedup vs baseline: 1.4512x; 1.0646x over previous
"""Trainium2 Bass kernel for nn_Attention_65223373357517.

Computes, for s,q [B=16, L=1024, D=1024] (D = 2H, H=512):
    a  = einsum('bsd,btd->bst', s, q)
    b  = softmax(a, -1) @ q
    c  = softmax(a^T, -1) @ s
    s~ = heuristic(s, b);  q~ = heuristic(q, c)
with heuristic(x, y) = g*r + (1-g)*x,
    r = gelu_tanh([x, y, x*y, x-y] @ w_r.T + b_r)
    g = sigmoid ([x, y, x*y, x-y] @ w_g.T + b_g)

Strategy: pure data-parallel over batch (2 examples per NeuronCore, 8 cores,
no collectives). Host folds the (x-y) block into the x/y weight blocks
(W1+W4, W2-W4, W3), transposes activations so every on-chip matmul is in
its natural layout, and transposes outputs back.  Masks are all-ones in
this problem configuration (additive mask term is identically zero), so
they do not enter the computation.

On-chip per batch (v2 schedule — PE kept saturated):
  S1:  A = S Q^T via f32r matmuls.  qt rows resident (prefetched during
       the previous batch's heuristic); st streamed as [128,128]
       stationary chunks, each reused for both 512-halves.  ms processed
       in waves of 3 (PSUM bound), k inner so compute chases DMA.
       Per A tile: row max m1, d1 = sum exp(A-m1) (fused ACT accum),
       l1 = m1 + ln d1 folded into the ACT bias so P1 = exp(A - l1) is
       the *normalized* softmax in A layout (bf16, per-partition bias —
       no cross-partition broadcast needed).  PE bf16-transposes P1 into
       p1t strips, issued one wave late so the ACT chain stays hidden.
  B/C: A^T tiles via PE fp32 transposes (PSUM); same per-tile stat chain
       gives P2^T = exp(A^T - l2) bf16, transposed back into p2 strips.
       b^T = Q_nat-chunk-contracted matmuls with rhs p1t; c^T likewise
       with lhsT = S_nat chunks, rhs p2.  at/p2 transposes interleave
       with the b^T matmul stream so PE never waits on stat chains.
  H:   heuristic per 128-row output strip: 24 K-chunk bf16 matmuls each
       for r and g over blocks [x^T, y^T, (x*y)^T]; gelu/sigmoid read
       PSUM with per-partition bias; epilogue out = x + g*(r - x);
       stream out s~^T / q~^T.  Next batch's qt prefetches here.
"""

import numpy as np
import ml_dtypes

B, L, D = 16, 1024, 1024
NCORES = 8
BLOC = B // NCORES          # batches per core
NK = D // 128               # contraction chunks (8)
NM = D // 128               # output-row chunks (8)
KF = 3 * D // 128           # folded heuristic contraction chunks (24)
NH = 2                      # 512-wide halves of a 1024 free dim

_nc_cache = None


def _build():
    import concourse.tile as tile
    from concourse import bacc, mybir

    FP32 = mybir.dt.float32
    FP32R = mybir.dt.float32r
    BF16 = mybir.dt.bfloat16
    AF = mybir.ActivationFunctionType
    ALU = mybir.AluOpType
    AX = mybir.AxisListType

    nc = bacc.Bacc("TRN2", target_bir_lowering=False, debug=False)

    st_d = nc.dram_tensor("st", [BLOC, D, L], FP32R, kind="ExternalInput")
    qt_d = nc.dram_tensor("qt", [BLOC, D, L], FP32R, kind="ExternalInput")
    snb_d = nc.dram_tensor("snb", [BLOC, L, D], BF16, kind="ExternalInput")
    qnb_d = nc.dram_tensor("qnb", [BLOC, L, D], BF16, kind="ExternalInput")
    stb_d = nc.dram_tensor("stb", [BLOC, D, L], BF16, kind="ExternalInput")
    qtb_d = nc.dram_tensor("qtb", [BLOC, D, L], BF16, kind="ExternalInput")
    F8 = mybir.dt.float8e4
    wrb_d = nc.dram_tensor("wrb", [NM, 128, 2 * NK, 128], BF16,
                           kind="ExternalInput")
    wr8_d = nc.dram_tensor("wr8", [NM, 128, 4, 2, 128], F8,
                           kind="ExternalInput")
    wgx_d = nc.dram_tensor("wgx", [NM, 128, NK, 128], BF16,
                           kind="ExternalInput")
    wg8_d = nc.dram_tensor("wg8", [NM, 128, NK, 2, 128], F8,
                           kind="ExternalInput")
    brt_d = nc.dram_tensor("brt", [128, NM], FP32, kind="ExternalInput")
    bgt_d = nc.dram_tensor("bgt", [128, NM], FP32, kind="ExternalInput")
    outs_d = nc.dram_tensor("outs", [BLOC, D, L], FP32, kind="ExternalOutput")
    outq_d = nc.dram_tensor("outq", [BLOC, D, L], FP32, kind="ExternalOutput")
    ident_d = nc.inline_tensor(np.eye(128, dtype=np.float32), name="identsrc")
    identb_d = nc.inline_tensor(
        np.eye(128, dtype=ml_dtypes.bfloat16), name="identbsrc")

    WAVES = [(0, 1, 2), (3, 4, 5), (6, 7)]

    with tile.TileContext(nc) as tc:
        # consumption order of (k, ms) stationary chunks across S1 waves
        ORDER = [(k, ms) for wave in WAVES for k in range(NK) for ms in wave]
        PRE = 8   # stf ring depth = cross-batch prefetch count
        NQP = 8   # qt row tiles per batch (full ring: read in every wave)

        with (
            tc.tile_pool(name="prog", bufs=1) as Pp,
            tc.tile_pool(name="qpool", bufs=1) as Pq,
            tc.tile_pool(name="stpool", bufs=1) as Pst,
        ):
            ident = Pp.tile([128, 128], FP32, tag="ident", name="ident")
            nc.sync.dma_start(ident[:], ident_d[:])
            identb = Pp.tile([128, 128], BF16, tag="identb", name="identb")
            nc.sync.dma_start(identb[:], identb_d[:])
            brt = Pp.tile([128, NM], FP32, tag="brt", name="brt")
            nc.sync.dma_start(brt[:], brt_d[:])
            bgt = Pp.tile([128, NM], FP32, tag="bgt", name="bgt")
            nc.sync.dma_start(bgt[:], bgt_d[:])

            def qtf_dma(b, k):
                t = Pq.tile([128, L], FP32R, tag="qtf", bufs=NQP,
                            name=f"qtf{b}_{k}")
                nc.sync.dma_start(t[:], qt_d[b, k * 128:(k + 1) * 128, :])
                return t

            def stf_dma(b, k, ms):
                t = Pst.tile([128, 128], FP32R, tag="stf", bufs=PRE,
                             name=f"stf{b}_{k}_{ms}")
                nc.sync.dma_start(
                    t[:], st_d[b, k * 128:(k + 1) * 128,
                               ms * 128:(ms + 1) * 128])
                return t

            # batch 0 inputs: interleave qt rows with the first stationary
            # chunks so wave-0 matmuls chase DMA arrival
            qtf = []
            stf_pre = {}
            pi = 0
            for k in range(NK):
                qtf.append(qtf_dma(0, k))
                while pi < PRE and ORDER[pi][0] <= k + 1:
                    stf_pre[ORDER[pi]] = stf_dma(0, *ORDER[pi])
                    pi += 1

            for b in range(BLOC):
                with tc.tile_pool(name=f"batch{b}", bufs=1) as Pb:
                    stats = {}
                    for nm_ in ("negm1", "d1", "rd1",
                                "negm2", "d2", "rd2"):
                        stats[nm_] = Pb.tile([128, NK], FP32, tag=nm_,
                                             name=f"{nm_}{b}")
                    stbt = [Pb.tile([128, L], BF16, tag="stb", bufs=NK,
                                    name=f"stb{b}_{k}") for k in range(NK)]
                    qtbt = [Pb.tile([128, L], BF16, tag="qtb", bufs=NK,
                                    name=f"qtb{b}_{k}") for k in range(NK)]
                    bT = []
                    cT = []

                    with tc.tile_pool(name=f"bc{b}", bufs=1) as Pc:
                        A = [Pc.tile([128, L], FP32, tag="A", bufs=NK,
                                     name=f"A{b}_{ms}") for ms in range(NK)]
                        p1t = Pc.tile([128, NK, L], BF16, tag="p1t",
                                      name=f"p1t{b}")
                        p2 = Pc.tile([128, NK, L], BF16, tag="p2",
                                     name=f"p2{b}")

                        def chain1a(ms):
                            # row max + E1 = exp(A - m1) with d1 row-sums
                            nc.vector.tensor_reduce(
                                stats["negm1"][:, ms:ms + 1], A[ms][:], AX.X,
                                ALU.max, negate=True)
                            e1 = Pc.tile([128, L], BF16, tag="e1", bufs=4,
                                         name=f"e1_{b}_{ms}")
                            nc.scalar.activation(
                                e1[:], A[ms][:], AF.Exp,
                                bias=stats["negm1"][:, ms:ms + 1],
                                accum_out=stats["d1"][:, ms:ms + 1])
                            return e1

                        def chain1b(ms):
                            # P1 = E1 / d1 (normalized softmax, bf16)
                            p1s = Pc.tile([128, L], BF16, tag="p1s", bufs=3,
                                          name=f"p1s{b}_{ms}")
                            nc.vector.tensor_scalar_mul(
                                p1s[:], e1_tiles[ms][:],
                                stats["rd1"][:, ms:ms + 1])
                            return p1s

                        e1_tiles = {}
                        p1s_tiles = {}

                        def t_p1(ms, pool):
                            strip = pool.tile([128, NK, 128], BF16,
                                              tag="strip", bufs=1,
                                              name=f"strp1{b}_{ms}")
                            for mt in range(NK):
                                nc.tensor.transpose(
                                    strip[:, mt, :],
                                    p1s_tiles[ms][:, mt * 128:(mt + 1) * 128],
                                    identb[:])
                            nc.vector.tensor_copy(
                                p1t[:, 0:NK, ms * 128:(ms + 1) * 128],
                                strip[:])

                        # ---------- S1: A = S Q^T + row softmax ----------
                        with tc.tile_pool(name=f"ps1_{b}", bufs=1,
                                          space="PSUM") as PS1:
                            for wi, wave in enumerate(WAVES):
                                pa = {}
                                for ms in wave:
                                    pa[ms] = [
                                        PS1.tile([128, 512], FP32, tag="pa",
                                                 bufs=7,
                                                 name=f"pa{b}_{ms}_{h}")
                                        for h in range(NH)]
                                for k in range(NK):
                                    for ms in wave:
                                        stf = stf_pre.pop(
                                            (k, ms), None) or stf_dma(b, k, ms)
                                        for h in range(NH):
                                            nc.tensor.matmul(
                                                pa[ms][h][:], stf[:],
                                                qtf[k][:,
                                                       h * 512:(h + 1) * 512],
                                                start=(k == 0),
                                                stop=(k == NK - 1))
                                for ms in wave:
                                    for h in range(NH):
                                        nc.vector.tensor_copy(
                                            A[ms][:, h * 512:(h + 1) * 512],
                                            pa[ms][h][:])
                                if wi > 0:
                                    pw = WAVES[wi - 1]
                                    nc.vector.reciprocal(
                                        stats["rd1"][:, pw[0]:pw[-1] + 1],
                                        stats["d1"][:, pw[0]:pw[-1] + 1])
                                    for ms in pw:
                                        p1s_tiles[ms] = chain1b(ms)
                                        t_p1(ms, PS1)
                                for ms in wave:
                                    e1_tiles[ms] = chain1a(ms)

                        # ---------- B/C: A^T softmax + stage-2 ----------
                        with tc.tile_pool(name=f"psbc_{b}", bufs=1,
                                          space="PSUM") as PSb:
                            at = {}

                            def at_mk(mt):
                                t = PSb.tile([128, L], FP32, tag="at", bufs=2,
                                             name=f"at{b}_{mt}")
                                for c in range(NK):
                                    nc.tensor.transpose(
                                        t[:, c * 128:(c + 1) * 128],
                                        A[c][:, mt * 128:(mt + 1) * 128],
                                        ident[:])
                                at[mt] = t

                            def chain2a(mt):
                                nc.vector.tensor_reduce(
                                    stats["negm2"][:, mt:mt + 1], at[mt][:],
                                    AX.X, ALU.max, negate=True)
                                e2 = Pc.tile([128, L], BF16, tag="e1",
                                             bufs=4, name=f"e2_{b}_{mt}")
                                nc.scalar.activation(
                                    e2[:], at[mt][:], AF.Exp,
                                    bias=stats["negm2"][:, mt:mt + 1],
                                    accum_out=stats["d2"][:, mt:mt + 1])
                                return e2

                            def chain2b(mt):
                                p2s = Pc.tile([128, L], BF16, tag="p2s",
                                              bufs=3, name=f"p2s{b}_{mt}")
                                nc.vector.tensor_scalar_mul(
                                    p2s[:], e2_tiles[mt][:],
                                    stats["rd2"][:, mt:mt + 1])
                                return p2s

                            e2_tiles = {}
                            p2s_tiles = {}

                            def t_p2(mt):
                                strip = PSb.tile([128, NK, 128], BF16,
                                                 tag="strip", bufs=1,
                                                 name=f"strp2{b}_{mt}")
                                for ms in range(NK):
                                    nc.tensor.transpose(
                                        strip[:, ms, :],
                                        p2s_tiles[mt][:,
                                                      ms * 128:(ms + 1) * 128],
                                        identb[:])
                                nc.vector.tensor_copy(
                                    p2[:, 0:NK, mt * 128:(mt + 1) * 128],
                                    strip[:])

                            # warm-up: first two at tiles + S1 tail
                            # transposes (their ACT chains need the gap)
                            lw = WAVES[-1]
                            at_mk(0)
                            nc.vector.reciprocal(
                                stats["rd1"][:, lw[0]:lw[-1] + 1],
                                stats["d1"][:, lw[0]:lw[-1] + 1])
                            p1s_tiles[lw[0]] = chain1b(lw[0])
                            t_p1(lw[0], PSb)
                            at_mk(1)
                            p1s_tiles[lw[1]] = chain1b(lw[1])
                            t_p1(lw[1], PSb)
                            e2_tiles[0] = chain2a(0)
                            e2_tiles[1] = chain2a(1)

                            def bmm(md, nat_d, pmov, out_list, tagc):
                                pb = [PSb.tile([128, 512], FP32, tag="pb",
                                               bufs=3,
                                               name=f"pb{b}_{tagc}{md}_{h}")
                                      for h in range(NH)]
                                for kt in range(NK):
                                    ch = Pc.tile(
                                        [128, 128], BF16, tag=f"{tagc}chunk",
                                        bufs=16, name=f"{tagc}ch{b}_{md}_{kt}")
                                    nc.sync.dma_start(
                                        ch[:],
                                        nat_d[b, kt * 128:(kt + 1) * 128,
                                              md * 128:(md + 1) * 128])
                                    for h in range(NH):
                                        nc.tensor.matmul(
                                            pb[h][:], ch[:],
                                            pmov[:, kt, h * 512:(h + 1) * 512],
                                            start=(kt == 0),
                                            stop=(kt == NK - 1))
                                ot = Pb.tile([128, L], BF16, tag=f"{tagc}T",
                                             bufs=NM, name=f"{tagc}T{b}_{md}")
                                for h in range(NH):
                                    nc.vector.tensor_copy(
                                        ot[:, h * 512:(h + 1) * 512], pb[h][:])
                                out_list.append(ot)

                            w0 = {}
                            for md in range(NM):
                                bmm(md, qnb_d, p1t, bT, "b")
                                if md == 1:
                                    # prefetch first heuristic g weights
                                    t = Pb.tile([128, NK, 128], BF16,
                                                tag="wgx0", name=f"wgx0_{b}")
                                    nc.sync.dma_start(t[:], wgx_d[0])
                                    w0["wgx"] = t
                                    t = Pb.tile([128, NK, 2, 128], F8,
                                                tag="wg80", name=f"wg80_{b}")
                                    nc.sync.dma_start(t[:], wg8_d[0])
                                    w0["wg8"] = t
                                if md >= 1:
                                    mt_t = md - 1
                                    if mt_t % 2 == 0:
                                        nc.vector.reciprocal(
                                            stats["rd2"][:, mt_t:mt_t + 2],
                                            stats["d2"][:, mt_t:mt_t + 2])
                                    p2s_tiles[mt_t] = chain2b(mt_t)
                                    t_p2(mt_t)
                                mt = md + 2
                                if mt < NK:
                                    at_mk(mt)
                                    e2_tiles[mt] = chain2a(mt)
                            p2s_tiles[NK - 1] = chain2b(NK - 1)
                            t_p2(NK - 1)
                            for k in range(NK):
                                nc.sync.dma_start(
                                    stbt[k][:],
                                    stb_d[b, k * 128:(k + 1) * 128, :])
                                nc.sync.dma_start(
                                    qtbt[k][:],
                                    qtb_d[b, k * 128:(k + 1) * 128, :])
                            for md in range(NM):
                                bmm(md, snb_d, p2, cT, "c")

                    # ---------- H: heuristic ----------
                    with (
                        tc.tile_pool(name=f"heur{b}", bufs=1) as Ph,
                        tc.tile_pool(name=f"psH{b}", bufs=7,
                                     space="PSUM") as PSh,
                    ):
                        # fp8 DoubleRow pair tiles for the g branch
                        # (j 0..3 = y chunks 2j/2j+1, j 4..7 = x*y chunks);
                        # side-s y pairs first: the first g matmuls need them
                        xys = []
                        xyq = []
                        f8s = []
                        f8q = []
                        for j in range(4):
                            t = Ph.tile([128, 2, L], F8, tag="f8s", bufs=NK,
                                        name=f"f8s{b}_{j}")
                            nc.vector.tensor_copy(t[:, 0, :], bT[2 * j][:])
                            nc.vector.tensor_copy(t[:, 1, :], bT[2 * j + 1][:])
                            f8s.append(t)
                        for k in range(NK):
                            t1 = Ph.tile([128, L], BF16, tag="xys", bufs=NK,
                                         name=f"xys{b}_{k}")
                            nc.vector.tensor_mul(t1[:], stbt[k][:], bT[k][:])
                            xys.append(t1)
                        for j in range(4):
                            t = Ph.tile([128, 2, L], F8, tag="f8s", bufs=NK,
                                        name=f"f8s{b}_{4 + j}")
                            nc.vector.tensor_copy(t[:, 0, :], xys[2 * j][:])
                            nc.vector.tensor_copy(t[:, 1, :],
                                                  xys[2 * j + 1][:])
                            f8s.append(t)
                        for j in range(4):
                            t = Ph.tile([128, 2, L], F8, tag="f8q", bufs=NK,
                                        name=f"f8q{b}_{j}")
                            nc.vector.tensor_copy(t[:, 0, :], cT[2 * j][:])
                            nc.vector.tensor_copy(t[:, 1, :], cT[2 * j + 1][:])
                            f8q.append(t)
                        for k in range(NK):
                            t2 = Ph.tile([128, L], BF16, tag="xyq", bufs=NK,
                                         name=f"xyq{b}_{k}")
                            nc.vector.tensor_mul(t2[:], qtbt[k][:], cT[k][:])
                            xyq.append(t2)
                        for j in range(4):
                            t = Ph.tile([128, 2, L], F8, tag="f8q", bufs=NK,
                                        name=f"f8q{b}_{4 + j}")
                            nc.vector.tensor_copy(t[:, 0, :], xyq[2 * j][:])
                            nc.vector.tensor_copy(t[:, 1, :],
                                                  xyq[2 * j + 1][:])
                            f8q.append(t)

                        for m in range(NM):
                            wrt = Ph.tile([128, 2 * NK, 128], BF16, tag="wr",
                                          bufs=2, name=f"wrt{b}_{m}")
                            nc.sync.dma_start(wrt[:], wrb_d[m])
                            wr8t = Ph.tile([128, 4, 2, 128], F8, tag="wr8",
                                           bufs=2, name=f"wr8t{b}_{m}")
                            nc.sync.dma_start(wr8t[:], wr8_d[m])
                            if m == 0:
                                wgxt, wg8t = w0["wgx"], w0["wg8"]
                            else:
                                wgxt = Ph.tile([128, NK, 128], BF16,
                                               tag="wgx", bufs=2,
                                               name=f"wgx{b}_{m}")
                                nc.sync.dma_start(wgxt[:], wgx_d[m])
                                wg8t = Ph.tile([128, NK, 2, 128], F8,
                                               tag="wg8", bufs=2,
                                               name=f"wg8{b}_{m}")
                                nc.sync.dma_start(wg8t[:], wg8_d[m])
                            if m == 0 and b + 1 < BLOC:
                                qtf = [qtf_dma(b + 1, k) for k in range(NQP)]
                                for i in range(PRE):
                                    stf_pre[ORDER[i]] = stf_dma(
                                        b + 1, *ORDER[i])
                            for xt, blocks, f8p, outd in (
                                (stbt, (stbt, bT, xys), f8s, outs_d),
                                (qtbt, (qtbt, cT, xyq), f8q, outq_d),
                            ):
                                tag = "s" if outd is outs_d else "q"
                                pg = [PSh.tile([128, 512], FP32, tag="rg",
                                               bufs=7,
                                               name=f"pg{b}_{m}{tag}{h}")
                                      for h in range(NH)]
                                pr = [PSh.tile([128, 512], FP32, tag="rg",
                                               bufs=7,
                                               name=f"pr{b}_{m}{tag}{h}")
                                      for h in range(NH)]
                                # g branch: x block bf16, y/x*y fp8 DoubleRow
                                for kf in range(NK):
                                    rhs = blocks[0][kf]
                                    for h in range(NH):
                                        nc.tensor.matmul(
                                            pg[h][:], wgxt[:, kf, :],
                                            rhs[:, h * 512:(h + 1) * 512],
                                            start=(kf == 0), stop=False)
                                for j in range(NK):
                                    for h in range(NH):
                                        nc.tensor.matmul(
                                            pg[h][:], wg8t[:, j, :, :],
                                            f8p[j][:, :,
                                                   h * 512:(h + 1) * 512],
                                            start=False, stop=(j == NK - 1),
                                            perf_mode=(
                                                mybir.MatmulPerfMode
                                                .DoubleRow))
                                # r branch: all bf16; sigmoid overlaps
                                g_sb = Ph.tile([128, L], BF16, tag="gsb",
                                               bufs=1, name=f"gsb{b}_{m}{tag}")
                                for h in range(NH):
                                    nc.scalar.activation(
                                        g_sb[:, h * 512:(h + 1) * 512],
                                        pg[h][:], AF.Sigmoid,
                                        bias=bgt[:, m:m + 1])
                                r_sb = Ph.tile([128, L], BF16, tag="rsb",
                                               bufs=1, name=f"rsb{b}_{m}{tag}")
                                t1 = Ph.tile([128, L], FP32, tag="t1", bufs=1,
                                             name=f"t1{b}_{m}{tag}")
                                t2 = Ph.tile([128, L], BF16, tag="t2", bufs=1,
                                             name=f"t2{b}_{m}{tag}")
                                osb = Ph.tile([128, L], FP32, tag="osb",
                                              bufs=1, name=f"osb{b}_{m}{tag}")
                                split = (m == NM - 1)
                                hgroups = ([(0,), (1,)] if split
                                           else [(0, 1)])
                                for hg in hgroups:
                                    for kf in range(2 * NK):
                                        rhs = blocks[kf // NK][kf % NK]
                                        for h in hg:
                                            nc.tensor.matmul(
                                                pr[h][:], wrt[:, kf, :],
                                                rhs[:, h * 512:(h + 1) * 512],
                                                start=(kf == 0), stop=False)
                                    for j in range(4):
                                        for h in hg:
                                            nc.tensor.matmul(
                                                pr[h][:], wr8t[:, j, :, :],
                                                f8p[4 + j][
                                                    :, :,
                                                    h * 512:(h + 1) * 512],
                                                start=False, stop=(j == 3),
                                                perf_mode=(
                                                    mybir.MatmulPerfMode
                                                    .DoubleRow))
                                    for h in hg:
                                        sl = slice(h * 512, (h + 1) * 512)
                                        nc.scalar.activation(
                                            r_sb[:, sl], pr[h][:],
                                            AF.Gelu_apprx_tanh,
                                            bias=brt[:, m:m + 1])
                                        nc.vector.tensor_sub(
                                            t1[:, sl], r_sb[:, sl],
                                            xt[m][:, sl])
                                        nc.vector.tensor_mul(
                                            t2[:, sl], g_sb[:, sl], t1[:, sl])
                                        nc.vector.tensor_add(
                                            osb[:, sl], t2[:, sl],
                                            xt[m][:, sl])
                                        nc.sync.dma_start(
                                            outd[b, m * 128:(m + 1) * 128,
                                                 sl], osb[:, sl])

    nc.compile()
    return nc


def _get_nc():
    global _nc_cache
    if _nc_cache is None:
        _nc_cache = _build()
    return _nc_cache


def _prep_inputs(s, q, w_r, b_r, w_g, b_g):
    bf = ml_dtypes.bfloat16
    s = np.ascontiguousarray(np.asarray(s, dtype=np.float32))
    q = np.ascontiguousarray(np.asarray(q, dtype=np.float32))
    w_r = np.asarray(w_r, dtype=np.float32)
    w_g = np.asarray(w_g, dtype=np.float32)
    b_r = np.asarray(b_r, dtype=np.float32)
    b_g = np.asarray(b_g, dtype=np.float32)

    st = np.ascontiguousarray(s.transpose(0, 2, 1))
    qt = np.ascontiguousarray(q.transpose(0, 2, 1))
    snb = s.astype(bf)
    qnb = q.astype(bf)
    stb = st.astype(bf)
    qtb = qt.astype(bf)

    def fold_w(w):
        W1, W2, W3, W4 = (w[:, i * D:(i + 1) * D] for i in range(4))
        eff = np.concatenate([W1 + W4, W2 - W4, W3], axis=1)  # [D, 3D]
        return eff.T  # [3D, D] contraction-major

    f8 = ml_dtypes.float8_e4m3
    wt_r = fold_w(w_r)
    wrb_pack = np.ascontiguousarray(
        wt_r[0:2 * D].reshape(2 * NK, 128, NM, 128)
        .transpose(2, 1, 0, 3)).astype(bf)
    wr8_pack = np.ascontiguousarray(
        wt_r[2 * D:3 * D].reshape(4, 2, 128, NM, 128)
        .transpose(3, 2, 0, 1, 4)).astype(f8)

    wt_g = fold_w(w_g)
    # g: x block bf16 [m, f, k, o]; y/x*y blocks fp8 DoubleRow pairs
    # [m, f, j, i, o] with contraction chunk = 8 + 2j + i
    wgx_pack = np.ascontiguousarray(
        wt_g[0:D].reshape(NK, 128, NM, 128).transpose(2, 1, 0, 3)).astype(bf)
    wg8_pack = np.ascontiguousarray(
        wt_g[D:3 * D].reshape(NK, 2, 128, NM, 128)
        .transpose(3, 2, 0, 1, 4)).astype(f8)
    brt = np.ascontiguousarray(b_r.reshape(NM, 128).T)
    bgt = np.ascontiguousarray(b_g.reshape(NM, 128).T)

    in_maps = []
    for c in range(NCORES):
        sl = slice(BLOC * c, BLOC * (c + 1))
        in_maps.append({
            "st": st[sl], "qt": qt[sl],
            "snb": snb[sl], "qnb": qnb[sl],
            "stb": stb[sl], "qtb": qtb[sl],
            "wrb": wrb_pack, "wr8": wr8_pack,
            "wgx": wgx_pack, "wg8": wg8_pack,
            "brt": brt, "bgt": bgt,
        })
    return in_maps


def run(inputs, trace=False, tmpdir=None):
    """Execute on 8 NeuronCores; returns ((s_tilde, q_tilde), BassKernelResults)."""
    from concourse.bass_utils import run_bass_kernel_spmd

    in_maps = _prep_inputs(
        inputs["s"], inputs["q"], inputs["w_r"], inputs["b_r"],
        inputs["w_g"], inputs["b_g"])
    nc = _get_nc()
    res = run_bass_kernel_spmd(nc, in_maps, list(range(NCORES)), trace=trace,
                               tmpdir=tmpdir)
    s_t = np.empty((B, L, D), np.float32)
    q_t = np.empty((B, L, D), np.float32)
    for c in range(NCORES):
        sl = slice(BLOC * c, BLOC * (c + 1))
        s_t[sl] = res.results[c]["outs"].transpose(0, 2, 1)
        q_t[sl] = res.results[c]["outq"].transpose(0, 2, 1)
    return (s_t, q_t), res


def kernel(s, q, w_r, b_r, w_g, b_g, s_mask=None, q_mask=None):
    # s_mask / q_mask are all-ones in this problem; the additive mask term
    # (1 - m1*m2) * NEG_INF is identically zero, so they are unused.
    out, _ = run({"s": s, "q": q, "w_r": w_r, "b_r": b_r,
                  "w_g": w_g, "b_g": b_g})
    return out
